# revision 16
# baseline (speedup 1.0000x reference)
"""AdaptiveSoftmaxProductHead.loss on 8 TRN2 NeuronCores (data-parallel).

Strategy
--------
Per-sample target log-prob = (head target logit - head logsumexp)
                           + [cluster: tail target logit - tail cluster logsumexp].

Host: assigns each of the 2048 samples to one of 8 cores, into one of two
128-slot tiles per core (tile A: cluster-2 + shortlist + cluster-0 overflow;
tile B: cluster-1 + cluster-0).  Gathers the per-sample target weight rows on
the host (pure data movement) so the device never needs data-dependent
indexing.  All device inputs are packed on the host into the exact SBUF
layout so each resident tensor loads with one (or few) large DMAs.

Device (identical SPMD program on every core, different data):
  - tail cluster logits [128 slots, osz] in 512-col PSUM chunks; 4 PE
    row-tiles (small-K packing) fill a 4-bank PSUM group.  Two tail clusters
    share one 128-slot tile via zero-masked stationaries and PSUM accumulate.
  - exp + per-slot row-sum of each [128, 2048] group is SPLIT between two
    engines: the ACT engine (true Exp with fused accumulator) and the DVE
    (Schraudolph bf16 exp: n = int16(z*128/ln2 + 16256); bitcast(n) ~ c*e^z,
    summed by a 4x-mode tensor_scalar with accum_out; the constant bias c
    is divided out in the same instruction).  This nearly doubles exp-sum
    throughput since the ACT engine alone was the critical resource.
  - head logits + exp-sums on ACT; target logits via per-slot dot products
    (VectorE products on GPSIMD + partition-dim ones-matmul on the PE).
  - ln + combine on device; host only unpermutes / adds the two parts.
"""

import numpy as np

# ---------------- problem constants (hardcoded; kernel.py is self-contained) ----
N, D = 2048, 512
SHORT = 1000
CUT = [1000, 10000, 50000, 100000]
OSZ = [9000, 40000, 50000]
HSZ = [128, 32, 8]
NCORES = 8
NSLOT = 128          # slots per tile
GRP = 2048           # columns per exp instruction (4 PSUM banks)
P0, P1, P2 = 10240, 40960, 51200   # padded tail column counts
PH = 1024                          # padded head columns
G0, G1, G2 = P0 // GRP, P1 // GRP, P2 // GRP   # 5, 20, 25 exp groups
Q1, Q2 = P1 // 4, P2 // 4          # per-quarter cols: 10240, 12800
ACC_COLS = 32
# w2_2 quarter chunks (in exp groups of 512 cols), group-aligned.
# chunk 0 = the G0 cluster-0-overlay groups (processed LAST in ring A);
# chunk 1 is small so the ring can start early.
W22_G = [5, 2, 4, 4, 5, 5]         # per-chunk group counts (sum = 25)
W21_CH = [10 * 512, 10 * 512]      # 5120, 5120
W20_CH = [3 * GRP, 2 * GRP]        # 6144, 4096

# bf16 Schraudolph exp on the DVE: n = int16(A16*z + B16), bitcast bf16.
# Mean multiplicative bias c (vs true e^z) is distribution-independent to
# ~2e-5; we divide it out in the reduce pass.  Calibrated midway between
# float->int truncation (c=1.037895) and round-to-nearest (c=1.040685)
# since the HW conversion mode costs at most 0.13% either way (harmless:
# it cancels to ~1e-4 relative in the final log-prob).
A16 = 184.66280009437495           # 128 / ln(2)
# 127*128, minus the bias correction folded in log-domain:
# 128*log2(1.0392913) = 7.11772
B16C = 16248.88228
KDVE = 14                          # exp groups handled by the DVE

# packed layout of the per-core "small" input tensor (bf16 columns)
OFF_W12, OFF_XA2 = 0, 32
OFF_W11, OFF_XB1 = 544, 672
OFF_W10, OFF_XA0, OFF_XB0 = 1184, 1696, 2208
OFF_XT, OFF_GRT, OFF_G2T, OFF_ONE = 2720, 3744, 4768, 5280
SMALL_C = 5296

_BUILT = None   # compiled Bass singleton


def _dve_groups():
    """Evenly spread KDVE of the 45 exp groups (consumption order) to the DVE.
    The first group and the two partial (column-trimmed) groups stay on ACT."""
    seq = [("A", t) for t in list(range(G0, G2)) + list(range(G0))] + \
          [("B", t) for t in range(G1)]
    partial = {("A", G2 - 1), ("B", G1 - 1)}
    elig = [i for i, s in enumerate(seq) if s not in partial and i != 0]
    idx = {elig[round(j * (len(elig) - 1) / (KDVE - 1))] for j in range(KDVE)}
    return {seq[i] for i in idx}


DVE_SET = _dve_groups()


# ================================ host-side prep ================================

def _assign(targets):
    """Per-core slot lists (tileA, tileB), each length-128 of sample idx or -1."""
    t = targets
    cl = np.where(t < CUT[0], -1,
                  np.where(t < CUT[1], 0, np.where(t < CUT[2], 1, 2)))
    idx_sl = np.nonzero(cl == -1)[0].tolist()
    idx_c = {k: np.nonzero(cl == k)[0].tolist() for k in range(3)}

    tileB = [[] for _ in range(NCORES)]
    for j, i in enumerate(idx_c[1]):
        tileB[j % NCORES].append(i)
    assert all(len(b) <= NSLOT for b in tileB), "cluster-1 per-core overflow"
    c0_ovf = []
    c = 0
    for i in idx_c[0]:
        placed = False
        for d in range(NCORES):
            cc = (c + d) % NCORES
            if len(tileB[cc]) < NSLOT:
                tileB[cc].append(i)
                c = (cc + 1) % NCORES
                placed = True
                break
        if not placed:
            c0_ovf.append(i)

    tileA = [[] for _ in range(NCORES)]
    for j, i in enumerate(idx_c[2]):
        tileA[j % NCORES].append(i)
    assert all(len(a) <= NSLOT for a in tileA), "cluster-2 per-core overflow"
    c = 0
    for i in idx_sl + c0_ovf:
        placed = False
        for d in range(NCORES):
            cc = (c + d) % NCORES
            if len(tileA[cc]) < NSLOT:
                tileA[cc].append(i)
                c = (cc + 1) % NCORES
                placed = True
                break
        assert placed, "tile A overflow"
    for c in range(NCORES):
        tileA[c] += [-1] * (NSLOT - len(tileA[c]))
        tileB[c] += [-1] * (NSLOT - len(tileB[c]))
    return tileA, tileB, cl


def _kpack(mat):
    """[512, C] -> [128, 4*C] with col = k*C + j (K-chunk packing for matmul)."""
    C = mat.shape[1]
    return np.ascontiguousarray(
        mat.reshape(4, 128, C).transpose(1, 0, 2).reshape(128, 4 * C))


def _host_arrays(inputs, tileA, tileB, cl):
    import ml_dtypes
    bf16 = ml_dtypes.bfloat16

    x = np.asarray(inputs["user_repr"], np.float32)
    t = np.asarray(inputs["targets"]).astype(np.int64)
    head_w = np.asarray(inputs["head_w"], np.float32)
    w1 = [np.asarray(inputs[f"tail_w1_{k}"], np.float32) for k in range(3)]
    w2 = [np.asarray(inputs[f"tail_w2_{k}"], np.float32) for k in range(3)]

    def pad_T(w, cols):
        out = np.zeros((w.shape[1], cols), np.float32)
        out[:, : w.shape[0]] = w.T
        return out

    # ---- shared (replicated) weights, packed into SBUF layout ----
    # head: col = (f*4 + k)*512 so each 512-col F-chunk is DMA-contiguous
    hwT = _kpack(pad_T(head_w, PH)).reshape(128, 4, 2, 512)\
        .transpose(0, 2, 1, 3).reshape(128, 4096).copy().astype(bf16)
    w1p = [_kpack(np.ascontiguousarray(w1[s].T)) for s in range(3)]  # [128, 4*hsz]
    w2_0T = pad_T(w2[0], P0).astype(bf16)                      # [128, 10240]

    def quarters(w, qpad):
        # [osz, hsz] -> [4, hsz, qpad]: osz split into 4 equal quarters,
        # each zero-padded to qpad (so the pad sits at each quarter's END)
        hsz = w.shape[1]
        qr = w.shape[0] // 4
        out = np.zeros((4, hsz, qpad), np.float32)
        out[:, :, :qr] = w.T.reshape(hsz, 4, qr).transpose(1, 0, 2)
        return out

    w2_1p = np.ascontiguousarray(
        quarters(w2[1], Q1).reshape(128, Q1)).astype(bf16)     # [128, 10240]
    # w2_2 packed dense: row = 8q + r; DMAed into SBUF partitions 32q + r
    w2_2p = np.ascontiguousarray(
        quarters(w2[2], Q2).reshape(32, Q2)).astype(bf16)      # [32, 12800]

    in_maps = []
    for c in range(NCORES):
        slots = tileA[c] + tileB[c]
        xs = np.zeros((256, D), np.float32)
        gr = np.zeros((256, D), np.float32)
        for s, i in enumerate(slots):
            if i < 0:
                continue
            xs[s] = x[i]
            ci = cl[i]
            gr[s] = head_w[int(t[i])] if ci == -1 else head_w[SHORT + ci]
        mA2 = np.array([1.0 if (i >= 0 and cl[i] == 2) else 0.0 for i in tileA[c]], np.float32)
        mA0 = np.array([1.0 if (i >= 0 and cl[i] == 0) else 0.0 for i in tileA[c]], np.float32)
        mB1 = np.array([1.0 if (i >= 0 and cl[i] == 1) else 0.0 for i in tileB[c]], np.float32)
        mB0 = np.array([1.0 if (i >= 0 and cl[i] == 0) else 0.0 for i in tileB[c]], np.float32)
        xA, xB = xs[:128], xs[128:]
        # masked transposed x for the four tail stationaries, K-chunk packed:
        # each block [128, 512] with col = k*128 + slot
        xm_blk = {}
        for key, mvec, xt in (("A2", mA2, xA), ("A0", mA0, xA),
                              ("B1", mB1, xB), ("B0", mB0, xB)):
            xm_blk[key] = _kpack(np.ascontiguousarray((xt * mvec[:, None]).T))
        # grT: gathered head rows, transposed + K-chunk packed like xT
        grT = _kpack(np.ascontiguousarray(gr.T))                  # [128, 1024]
        # gathered tail target rows, transposed: [hsz, 128] in 4 col-blocks
        g2T = np.zeros((128, 4 * 128), np.float32)
        for s, i in enumerate(tileA[c]):
            if i < 0:
                continue
            if cl[i] == 2:
                g2T[:8, s] = w2[2][int(t[i]) - CUT[2]]
            elif cl[i] == 0:
                g2T[:, 128 + s] = w2[0][int(t[i]) - CUT[0]]
        for s, i in enumerate(tileB[c]):
            if i < 0:
                continue
            if cl[i] == 1:
                g2T[:32, 256 + s] = w2[1][int(t[i]) - CUT[1]]
            elif cl[i] == 0:
                g2T[:, 384 + s] = w2[0][int(t[i]) - CUT[0]]
        # pad corrections: the last ring group is trimmed to the real column
        # count, so cluster-2/1 rows see no zero-pad exps; cluster-0 rows
        # still see P0 - osz0 of them
        corrA = -(P0 - OSZ[0]) * mA0 + (1.0 - mA2 - mA0)
        corrB = -(P0 - OSZ[0]) * mB0 + (1.0 - mB1 - mB0)
        mcpack = np.stack([mA2, mA0, mB1, mB0, corrA, corrB], axis=1)\
            .astype(np.float32)                                   # [128, 6]

        # one packed small-input tensor, consumption-ordered so the first
        # DMA splits carry exactly what the first projections need
        small = np.concatenate([
            w1p[2],                                  # [128, 32]   OFF_W12
            xm_blk["A2"],                            # [128, 512]  OFF_XA2
            w1p[1],                                  # [128, 128]  OFF_W11
            xm_blk["B1"],                            # [128, 512]  OFF_XB1
            w1p[0],                                  # [128, 512]  OFF_W10
            xm_blk["A0"],                            # [128, 512]  OFF_XA0
            xm_blk["B0"],                            # [128, 512]  OFF_XB0
            _kpack(np.ascontiguousarray(xs.T)),      # [128, 1024] OFF_XT
            grT,                                     # [128, 1024] OFF_GRT
            g2T,                                     # [128, 512]  OFF_G2T
            np.ones((128, 16), np.float32),          # [128, 16]   OFF_ONE
        ], axis=1)
        m = {
            "small": small.astype(bf16),
            "mcpack": mcpack,
            "hwT": hwT,
            "w2_0T": w2_0T, "w2_1p": w2_1p, "w2_2p": w2_2p,
        }
        in_maps.append(m)
    return in_maps


# ================================ device program ================================

def build_nc():
    from concourse import bacc, tile
    import concourse.mybir as mybir

    bf = mybir.dt.bfloat16
    f32 = mybir.dt.float32
    i16 = mybir.dt.int16
    AF = mybir.ActivationFunctionType
    ALU = mybir.AluOpType
    AX = mybir.AxisListType

    nc = bacc.Bacc("TRN2", target_bir_lowering=False, debug=False,
                   num_devices=NCORES)

    dp = nc.declare_dram_parameter
    d_small = dp("small", [128, SMALL_C], bf, False)
    d_mc = dp("mcpack", [128, 6], f32, False)
    d_hwT = dp("hwT", [128, 4 * PH], bf, False)
    d_w2_0T = dp("w2_0T", [HSZ[0], P0], bf, False)
    d_w2_1p = dp("w2_1p", [128, Q1], bf, False)
    d_w2_2p = dp("w2_2p", [32, Q2], bf, False)
    d_out = dp("out", [128, 4], f32, True)

    with tile.TileContext(nc) as tc:
        with tc.tile_pool(name="consts", bufs=1) as cp, \
             tc.tile_pool(name="acc", bufs=1) as ap_, \
             tc.tile_pool(name="scr", bufs=2) as sp, \
             tc.tile_pool(name="iscr", bufs=2) as ipool, \
             tc.tile_pool(name="dscr", bufs=2) as dpool:

            # pin the table set that holds BOTH Exp and Ln so the tail Ln
            # does not pay a ~2.7us table switch
            nc.scalar.add_instruction(mybir.InstLoadActFuncSet(
                name=nc.get_next_instruction_name(), ins=[], outs=[],
                act_func_set_id=6))

            # ------------- DMA issue (sync queue), consumption-ordered --------
            s_small = cp.tile([128, SMALL_C], bf, name="s_small", tag="s_small")

            def sm_load(lo, hi):
                nc.sync.dma_start(out=s_small[:, lo:hi], in_=d_small.ap()[:, lo:hi])

            def w22_tile(ch):
                # 4 dense quarter-DMAs per chunk, issued on the (otherwise
                # idle) GPSIMD queue so they don't serialize behind the sync
                # queue's issue stream
                cw = W22_G[ch] * 512
                off = sum(W22_G[:ch]) * 512
                t_ = cp.tile([128, cw], bf, name=f"s_w22_{ch}", tag=f"s_w22_{ch}")
                for g in range(4):
                    nc.gpsimd.dma_start(
                        out=t_[32 * g:32 * g + 8, :],
                        in_=d_w2_2p.ap()[8 * g:8 * g + 8, off:off + cw])
                return t_

            def w20_tile(ch):
                cw = W20_CH[ch]
                off = W20_CH[0] if ch else 0
                t_ = cp.tile([128, cw], bf, name=f"s_w20_{ch}", tag=f"s_w20_{ch}")
                nc.sync.dma_start(out=t_[:, :], in_=d_w2_0T.ap()[:, off:off + cw])
                return t_

            def w21_tile(ch):
                cw = W21_CH[ch]
                off = W21_CH[0] if ch else 0
                t_ = cp.tile([128, cw], bf, name=f"s_w21_{ch}", tag=f"s_w21_{ch}")
                nc.sync.dma_start(out=t_[:, :], in_=d_w2_1p.ap()[:, off:off + cw])
                return t_

            s_w22 = [None] * 6
            sm_load(0, 544)              # w1_2 + xmA2  -> h2 projection
            sm_load(544, 1184)           # w1_1 + xmB1  -> h1 projection
            s_w22[1] = w22_tile(1)       # ring A t=5,6
            sm_load(1184, 2720)          # w1_0 + xmA0 + xmB0 -> h0 projection
            s_w22[2] = w22_tile(2)       # t=7..10
            sm_load(2720, 3744)          # xT -> heads
            s_hwT = cp.tile([128, 4096], bf, name="s_hwT", tag="s_hwT")
            nc.sync.dma_start(out=s_hwT[:, :], in_=d_hwT.ap())
            s_w22[3] = w22_tile(3)       # t=11..14
            s_w22[4] = w22_tile(4)       # t=15..19
            s_w22[5] = w22_tile(5)       # t=20..24
            s_w20 = [w20_tile(0), None]  # overlay t=0..2
            s_w22[0] = w22_tile(0)       # overlay t=0..4
            s_w20[1] = w20_tile(1)       # overlay t=3..4
            sm_load(3744, SMALL_C)       # grT + g2T + ones -> p-products, dots
            s_w21 = [w21_tile(0), w21_tile(1)]
            s_mc = cp.tile([128, 6], f32, name="s_mc", tag="s_mc")
            nc.sync.dma_start(out=s_mc[:, :], in_=d_mc.ap())

            # ------------- views ----------------------------------------------
            def xm(m, k):        # masked-x chunk [128, 128]: m in A2,A0,B1,B0
                base = {0: OFF_XA2, 1: OFF_XA0, 2: OFF_XB1, 3: OFF_XB0}[m]
                return s_small[:, base + k * 128: base + k * 128 + 128]

            def xTk(k, sl):      # xT chunk [128, 128] for slot range sl
                return s_small[:, OFF_XT + k * 256 + sl * 128:
                               OFF_XT + k * 256 + sl * 128 + 128]

            def hwk(k, f):       # head weight chunk [128, 512]
                o = (f * 4 + k) * 512
                return s_hwT[:, o: o + 512]

            def w1k(seg, k):     # w1 seg (0:128 | 1:32 | 2:8) k-chunk
                base = {0: OFF_W10, 1: OFF_W11, 2: OFF_W12}[seg]
                w = HSZ[seg]
                return s_small[:, base + k * w: base + (k + 1) * w]

            s_xT = s_small[:, OFF_XT:OFF_XT + 1024]
            s_grT = s_small[:, OFF_GRT:OFF_GRT + 1024]
            s_g2T = s_small[:, OFF_G2T:OFF_G2T + 512]
            s_ones = s_small[:, OFF_ONE:OFF_ONE + 1]

            mA2_v, mA0_v = s_mc[:, 0:1], s_mc[:, 1:2]
            mB1_v, mB0_v = s_mc[:, 2:3], s_mc[:, 3:4]
            corrA_v, corrB_v = s_mc[:, 4:5], s_mc[:, 5:6]

            # accumulators / combine tiles
            accA = ap_.tile([128, ACC_COLS], f32, name="accA", tag="accA")
            accB = ap_.tile([128, ACC_COLS], f32, name="accB", tag="accB")
            accH = ap_.tile([128, 4], f32, name="accH", tag="accH")
            tgt4 = ap_.tile([128, 4], f32, name="tgt4", tag="tgt4")
            S4 = ap_.tile([128, 4], f32, name="S4", tag="S4")
            ln4 = ap_.tile([128, 4], f32, name="ln4", tag="ln4")
            out4 = ap_.tile([128, 4], f32, name="out4", tag="out4")
            tmp = [ap_.tile([128, 1], f32, name=f"tmp{i}", tag=f"tmp{i}")
                   for i in range(6)]
            s_h2a = ap_.tile([128, 128], bf, name="s_h2a", tag="s_h2a")
            s_h1b = ap_.tile([128, 128], bf, name="s_h1b", tag="s_h1b")
            s_h0a = ap_.tile([128, 128], bf, name="s_h0a", tag="s_h0a")
            s_h0b = ap_.tile([128, 128], bf, name="s_h0b", tag="s_h0b")

            with tc.tile_pool(name="psR", bufs=2, space="PSUM") as pr:
                # ---- early projections as ring-pool pseudo-groups ----
                # h2 matmuls + CASTs first: they gate the whole ring.  The
                # h1 projection is emitted AFTER the first ring group so the
                # first exp's engine-counter dependency doesn't include it.
                h21 = pr.tile([128, 256], f32, name="h21", tag="ring")
                for g in range(4):
                    for k in range(4):
                        nc.tensor.matmul(
                            h21[32 * g:32 * g + 8, 0:128], w1k(2, k), xm(0, k),
                            start=(k == 0), stop=(k == 3),
                            tile_position=(0, 32 * g))
                for g in range(4):
                    nc.vector.tensor_copy(s_h2a[32 * g:32 * g + 8, :],
                                          h21[32 * g:32 * g + 8, 0:128])

                def h1_proj():
                    for g in range(4):
                        for k in range(4):
                            nc.tensor.matmul(
                                h21[32 * g:32 * g + 32, 128:256], w1k(1, k), xm(2, k),
                                start=(k == 0), stop=(k == 3),
                                tile_position=(0, 32 * g))
                    for g in range(4):
                        nc.vector.tensor_copy(s_h1b[32 * g:32 * g + 32, :],
                                              h21[32 * g:32 * g + 32, 128:256])

                # ---- exp + per-slot row sum, split ACT / DVE ----
                def exp_group(pt, acc_ap, dve, fw=512):
                    if dve:
                        assert fw == 512
                        it = ipool.tile([128, GRP], i16, name="it", tag="it")
                        nc.vector.tensor_scalar(
                            it[:, :], pt[:, :], A16, B16C, ALU.mult, ALU.add)
                        ds = dpool.tile([128, GRP], bf, name="ds", tag="ds")
                        nc.vector.tensor_scalar(
                            ds[:, :], it[:, :].bitcast(bf), 1.0, None,
                            ALU.mult, ALU.add, accum_out=acc_ap)
                    else:
                        rsc = sp.tile([128, GRP], bf, name="rscr", tag="rscr")
                        src = pt[:, :] if fw == 512 else \
                            pt.rearrange("p (b e) -> p b e", b=4)[:, :, 0:fw]
                        dst = rsc[:, :] if fw == 512 else \
                            rsc.rearrange("p (b e) -> p b e", b=4)[:, :, 0:fw]
                        nc.scalar.activation(dst, src, AF.Exp, accum_out=acc_ap)

                # ring A: cluster-2 quarters + cluster-0 overflow (groups < G0).
                # Groups >= G0 (no w2_0 dependency) run first so the ring can
                # start as soon as the first w2_2 chunk lands.
                w22_bounds = np.cumsum([0] + W22_G).tolist()

                def ringA_group(t):
                    ch = next(i for i in range(len(W22_G))
                              if w22_bounds[i + 1] > t)
                    w = t - w22_bounds[ch]
                    # last group: only 12500 % 512 = 212 real cols per quarter
                    fw = 212 if t == G2 - 1 else 512
                    pt = pr.tile([128, GRP], f32, name="ringA", tag="ring")
                    for g in range(4):
                        nc.tensor.matmul(
                            pt[:, g * 512:g * 512 + fw],
                            s_h2a[32 * g:32 * g + 8, :],
                            s_w22[ch][32 * g:32 * g + 8, w * 512:w * 512 + fw],
                            start=True, stop=(t >= G0),
                            tile_position=(32 * g, 0))
                    if t < G0:
                        ch0 = 0 if t < 3 else 1
                        o0 = t * GRP - (0 if ch0 == 0 else W20_CH[0])
                        for g in range(4):
                            nc.tensor.matmul(
                                pt[:, g * 512:(g + 1) * 512],
                                s_h0a[:, :],
                                s_w20[ch0][:, o0 + g * 512:o0 + (g + 1) * 512],
                                start=False, stop=True)
                    exp_group(pt, accA[:, t:t + 1], ("A", t) in DVE_SET, fw)

                def head_tile(s):
                    # head logits + exp-sum for sample tile s; one ring slot,
                    # PE cost hides under ring ACT slack
                    hp = pr.tile([128, PH], f32, name=f"head_ps{s}", tag="ring")
                    for f in range(PH // 512):
                        for k in range(4):
                            nc.tensor.matmul(
                                hp[:, f * 512:(f + 1) * 512],
                                xTk(k, s), hwk(k, f),
                                start=(k == 0), stop=(k == 3))
                    hsc = sp.tile([128, PH], bf, name="hscr", tag="rscr")
                    nc.scalar.activation(hsc[:, :], hp[:, :], AF.Exp,
                                         accum_out=accH[:, 2 * s:2 * s + 1])

                for t in range(G0, G2):
                    if t == 6:
                        h1_proj()
                    elif t == 7:
                        # h0 projections as one ring-pool pseudo-group (their
                        # inputs land in DMA 4; only overlay groups need them)
                        h0t = pr.tile([128, 256], f32, name="h0t", tag="ring")
                        for k in range(4):
                            nc.tensor.matmul(h0t[:, 0:128], w1k(0, k), xm(1, k),
                                             start=(k == 0), stop=(k == 3))
                        for k in range(4):
                            nc.tensor.matmul(h0t[:, 128:256], w1k(0, k), xm(3, k),
                                             start=(k == 0), stop=(k == 3))
                        nc.vector.tensor_copy(s_h0a[:, :], h0t[:, 0:128])
                        nc.vector.tensor_copy(s_h0b[:, :], h0t[:, 128:256])
                    ringA_group(t)
                    if t == 14:
                        head_tile(0)
                    elif t == 16:
                        head_tile(1)

                # p-products for the target dots (the dot matmuls themselves
                # run after ring B starts)
                ph = sp.tile([128, 1024], bf, name="ph", tag="ph")
                nc.vector.tensor_mul(ph[:, :], s_xT[:, :], s_grT[:, :])
                p2 = sp.tile([128, 128], bf, name="p2", tag="p2")
                nc.vector.tensor_mul(p2[0:8, :], s_h2a[0:8, :], s_g2T[0:8, 0:128])
                p0a = sp.tile([128, 128], bf, name="p0a", tag="p0a")
                nc.vector.tensor_mul(p0a[:, :], s_h0a[:, :], s_g2T[:, 128:256])
                p1 = sp.tile([128, 128], bf, name="p1", tag="p1")
                nc.vector.tensor_mul(p1[0:32, :], s_h1b[0:32, :], s_g2T[0:32, 256:384])
                p0b = sp.tile([128, 128], bf, name="p0b", tag="p0b")
                nc.vector.tensor_mul(p0b[:, :], s_h0b[:, :], s_g2T[:, 384:512])

                def ringB_group(t):
                    ch = t // 10
                    w = t % 10
                    # last group: only 10000 % 512 = 272 real cols per quarter
                    fw = 272 if t == G1 - 1 else 512
                    pt = pr.tile([128, GRP], f32, name="ringB", tag="ring")
                    for g in range(4):
                        nc.tensor.matmul(
                            pt[:, g * 512:g * 512 + fw],
                            s_h1b[32 * g:32 * g + 32, :],
                            s_w21[ch][32 * g:32 * g + 32, w * 512:w * 512 + fw],
                            start=True, stop=(t >= G0),
                            tile_position=(32 * g, 0))
                    if t < G0:
                        ch0 = 0 if t < 3 else 1
                        o0 = t * GRP - (0 if ch0 == 0 else W20_CH[0])
                        for g in range(4):
                            nc.tensor.matmul(
                                pt[:, g * 512:(g + 1) * 512],
                                s_h0b[:, :],
                                s_w20[ch0][:, o0 + g * 512:o0 + (g + 1) * 512],
                                start=False, stop=True)
                    exp_group(pt, accB[:, t:t + 1], ("B", t) in DVE_SET, fw)

                for t in range(G0):
                    ringA_group(t)

                # ring-A side of the combine (DVE; runs while ring B exps)
                nc.vector.tensor_reduce(tmp[0][:, :], accA[:, 0:G2], axis=AX.X, op=ALU.add)
                nc.vector.tensor_reduce(tmp[1][:, :], accA[:, 0:G0], axis=AX.X, op=ALU.add)
                nc.vector.tensor_mul(tmp[0][:, :], tmp[0][:, :], mA2_v)
                nc.vector.tensor_mul(tmp[1][:, :], tmp[1][:, :], mA0_v)
                nc.vector.tensor_add(tmp[4][:, :], tmp[0][:, :], tmp[1][:, :])
                nc.vector.tensor_add(S4[:, 2:3], tmp[4][:, :], corrA_v)
                nc.vector.tensor_scalar_add(S4[:, 0:1], accH[:, 0:1], float(-(PH - 1003)))
                nc.vector.tensor_scalar_add(S4[:, 1:2], accH[:, 2:3], float(-(PH - 1003)))

                for t in range(7):
                    ringB_group(t)

                # target-logit dots: per-slot dot(u, v) = (u .* v)^T @ ones
                # (partition-dim contraction on the PE -> [slots, 1] PSUM);
                # one slot in ring B's light region
                dots_ps = pr.tile([128, GRP], f32, name="dots_ps", tag="ring")
                for k in range(4):
                    nc.tensor.matmul(dots_ps[:, 0:1], ph[:, k * 256:k * 256 + 128],
                                     s_ones[:, :], start=(k == 0), stop=(k == 3))
                for k in range(4):
                    nc.tensor.matmul(dots_ps[:, 1:2],
                                     ph[:, k * 256 + 128:k * 256 + 256],
                                     s_ones[:, :], start=(k == 0), stop=(k == 3))
                nc.tensor.matmul(dots_ps[:, 2:3], p2[0:8, :], s_ones[0:8, :],
                                 start=True, stop=False)
                nc.tensor.matmul(dots_ps[:, 2:3], p0a[:, :], s_ones[:, :],
                                 start=False, stop=True)
                nc.tensor.matmul(dots_ps[:, 3:4], p1[0:32, :], s_ones[0:32, :],
                                 start=True, stop=False)
                nc.tensor.matmul(dots_ps[:, 3:4], p0b[:, :], s_ones[:, :],
                                 start=False, stop=True)
                nc.vector.tensor_copy(tgt4[:, :], dots_ps[:, 0:4])

                for t in range(7, G1):
                    ringB_group(t)

            # ------------- combine (only ring-B accB remains) ----------------
            nc.vector.tensor_reduce(tmp[2][:, :], accB[:, 0:G1], axis=AX.X, op=ALU.add)
            nc.vector.tensor_reduce(tmp[3][:, :], accB[:, 0:G0], axis=AX.X, op=ALU.add)
            nc.vector.tensor_mul(tmp[2][:, :], tmp[2][:, :], mB1_v)
            nc.vector.tensor_mul(tmp[3][:, :], tmp[3][:, :], mB0_v)
            nc.vector.tensor_add(tmp[5][:, :], tmp[2][:, :], tmp[3][:, :])
            nc.vector.tensor_add(S4[:, 3:4], tmp[5][:, :], corrB_v)
            nc.scalar.activation(ln4[:, :], S4[:, :], AF.Ln)
            nc.vector.tensor_sub(out4[:, :], tgt4[:, :], ln4[:, :])
            nc.sync.dma_start(out=d_out.ap(), in_=out4[:, :])

    nc.compile()
    return nc


def _get_nc():
    global _BUILT
    if _BUILT is None:
        _BUILT = build_nc()
    return _BUILT


# ================================ entry point ================================

def _numpy_fallback(inputs):
    """Last-resort exact computation (only if the slot assignment misfits,
    which cannot happen for the deterministic problem inputs)."""
    x = np.asarray(inputs["user_repr"], np.float64)
    t = np.asarray(inputs["targets"]).astype(np.int64)
    head_w = np.asarray(inputs["head_w"], np.float64)
    rows = np.arange(x.shape[0])

    def lse_rows(logits):
        m = logits.max(axis=1, keepdims=True)
        return (np.log(np.exp(logits - m).sum(axis=1, keepdims=True)) + m)

    hl = x @ head_w.T
    head_lp = hl - lse_rows(hl)
    out = np.where(t < SHORT, head_lp[rows, np.minimum(t, SHORT - 1)], 0.0)
    for i in range(3):
        w1 = np.asarray(inputs[f"tail_w1_{i}"], np.float64)
        w2 = np.asarray(inputs[f"tail_w2_{i}"], np.float64)
        tl = (x @ w1.T) @ w2.T
        tail_lp = tl - lse_rows(tl)
        rel = np.clip(t - CUT[i], 0, CUT[i + 1] - CUT[i] - 1)
        val = head_lp[:, SHORT + i] + tail_lp[rows, rel]
        out = np.where((t >= CUT[i]) & (t < CUT[i + 1]), val, out)
    return out.astype(np.float32)


def _logit_bound_ok(inputs):
    """Cauchy-Schwarz bound on |tail logit| so the int16 Schraudolph path
    cannot overflow (needs |z| < ~80; typical max is ~12)."""
    x = np.asarray(inputs["user_repr"], np.float32)
    bound = 0.0
    for k in range(3):
        w1 = np.asarray(inputs[f"tail_w1_{k}"], np.float32)
        w2 = np.asarray(inputs[f"tail_w2_{k}"], np.float32)
        h = x @ w1.T
        hn = np.sqrt((h * h).sum(axis=1)).max()
        wn = np.sqrt((w2 * w2).sum(axis=1)).max()
        bound = max(bound, float(hn) * float(wn))
    return bound < 60.0


def kernel(**inputs):
    from concourse.bass_utils import run_bass_kernel_spmd

    targets = np.asarray(inputs["targets"]).astype(np.int64)
    try:
        tileA, tileB, cl = _assign(targets)
        if not _logit_bound_ok(inputs):
            return _numpy_fallback(inputs)
    except AssertionError:
        return _numpy_fallback(inputs)
    in_maps = _host_arrays(inputs, tileA, tileB, cl)
    nc = _get_nc()
    res = run_bass_kernel_spmd(nc, in_maps, core_ids=list(range(NCORES)))
    out = np.zeros(N, np.float32)
    for c in range(NCORES):
        o = res.results[c]["out"]   # [128, 4]
        for s, i in enumerate(tileA[c]):
            if i >= 0:
                out[i] = o[s, 0] + (o[s, 2] if cl[i] >= 0 else 0.0)
        for s, i in enumerate(tileB[c]):
            if i >= 0:
                out[i] = o[s, 1] + o[s, 3]
    return out


# revision 22
# speedup vs baseline: 1.0158x; 1.0158x over previous
"""AdaptiveSoftmaxProductHead.loss on 8 TRN2 NeuronCores (data-parallel).

Strategy
--------
Per-sample target log-prob = (head target logit - head logsumexp)
                           + [cluster: tail target logit - tail cluster logsumexp].

Host: assigns each of the 2048 samples to one of 8 cores, into one of two
128-slot tiles per core (tile A: cluster-2 + shortlist + cluster-0 overflow;
tile B: cluster-1 + cluster-0).  Gathers the per-sample target weight rows on
the host (pure data movement) so the device never needs data-dependent
indexing.  All device inputs are packed on the host into the exact SBUF
layout so each resident tensor loads with one (or few) large DMAs.

Device (identical SPMD program on every core, different data):
  - tail cluster logits [128 slots, osz] in 512-col PSUM chunks; 4 PE
    row-tiles (small-K packing) fill a 4-bank PSUM group.  Two tail clusters
    share one 128-slot tile via zero-masked stationaries and PSUM accumulate.
  - exp + per-slot row-sum of each [128, 2048] group is SPLIT between two
    engines: the ACT engine (true Exp with fused accumulator) and the DVE
    (Schraudolph bf16 exp: n = int16(z*128/ln2 + 16256); bitcast(n) ~ c*e^z,
    summed by a 4x-mode tensor_scalar with accum_out; the constant bias c
    is divided out in the same instruction).  This nearly doubles exp-sum
    throughput since the ACT engine alone was the critical resource.
  - head logits + exp-sums on ACT; target logits via per-slot dot products
    (VectorE products on GPSIMD + partition-dim ones-matmul on the PE).
  - ln + combine on device; host only unpermutes / adds the two parts.
"""

import numpy as np

# ---------------- problem constants (hardcoded; kernel.py is self-contained) ----
N, D = 2048, 512
SHORT = 1000
CUT = [1000, 10000, 50000, 100000]
OSZ = [9000, 40000, 50000]
HSZ = [128, 32, 8]
NCORES = 8
NSLOT = 128          # slots per tile
GRP = 2048           # columns per exp instruction (4 PSUM banks)
P0, P1, P2 = 10240, 40960, 51200   # padded tail column counts
PH = 1024                          # padded head columns
G0, G1, G2 = P0 // GRP, P1 // GRP, P2 // GRP   # 5, 20, 25 exp groups
Q1, Q2 = P1 // 4, P2 // 4          # per-quarter cols: 10240, 12800
ACC_COLS = 32
# w2_2 quarter chunks (in exp groups of 512 cols), group-aligned.
# chunk 0 = the G0 cluster-0-overlay groups (processed LAST in ring A);
# chunk 1 is small so the ring can start early.
W22_G = [5, 2, 4, 4, 5, 5]         # per-chunk group counts (sum = 25)
W21_CH = [10 * 512, 10 * 512]      # 5120, 5120
W20_CH = [3 * GRP, 2 * GRP]        # 6144, 4096

# bf16 Schraudolph exp on the DVE: n = int16(A16*z + B16), bitcast bf16.
# Mean multiplicative bias c (vs true e^z) is distribution-independent to
# ~2e-5; we divide it out in the reduce pass.  Calibrated midway between
# float->int truncation (c=1.037895) and round-to-nearest (c=1.040685)
# since the HW conversion mode costs at most 0.13% either way (harmless:
# it cancels to ~1e-4 relative in the final log-prob).
A16 = 184.66280009437495           # 128 / ln(2)
# 127*128, minus the bias correction folded in log-domain:
# 128*log2(1.0392913) = 7.11772
B16C = 16248.88228
KDVE = 15                          # exp groups handled by the DVE

# packed layout of the per-core "small" input tensor (bf16 columns)
OFF_W12, OFF_XA2 = 0, 32
OFF_W11, OFF_XB1 = 544, 672
OFF_W10, OFF_XA0, OFF_XB0 = 1184, 1696, 2208
OFF_XT, OFF_GRT, OFF_G2T, OFF_ONE = 2720, 3744, 4768, 5280
SMALL_C = 5296

_BUILT = None   # compiled Bass singleton


def _dve_groups():
    """Evenly spread KDVE of the 45 exp groups (consumption order) to the DVE.
    The first group and the two partial (column-trimmed) groups stay on ACT."""
    seq = [("A", t) for t in list(range(G0, G2)) + list(range(G0))] + \
          [("B", t) for t in range(G1)]
    partial = {("A", G2 - 1), ("B", G1 - 1)}
    elig = [i for i, s in enumerate(seq) if s not in partial and i != 0]
    idx = {elig[round(j * (len(elig) - 1) / (KDVE - 1))] for j in range(KDVE)}
    return {seq[i] for i in idx}


DVE_SET = _dve_groups()


# ================================ host-side prep ================================

def _assign(targets):
    """Per-core slot lists (tileA, tileB), each length-128 of sample idx or -1."""
    t = targets
    cl = np.where(t < CUT[0], -1,
                  np.where(t < CUT[1], 0, np.where(t < CUT[2], 1, 2)))
    idx_sl = np.nonzero(cl == -1)[0].tolist()
    idx_c = {k: np.nonzero(cl == k)[0].tolist() for k in range(3)}

    tileB = [[] for _ in range(NCORES)]
    for j, i in enumerate(idx_c[1]):
        tileB[j % NCORES].append(i)
    assert all(len(b) <= NSLOT for b in tileB), "cluster-1 per-core overflow"
    c0_ovf = []
    c = 0
    for i in idx_c[0]:
        placed = False
        for d in range(NCORES):
            cc = (c + d) % NCORES
            if len(tileB[cc]) < NSLOT:
                tileB[cc].append(i)
                c = (cc + 1) % NCORES
                placed = True
                break
        if not placed:
            c0_ovf.append(i)

    tileA = [[] for _ in range(NCORES)]
    for j, i in enumerate(idx_c[2]):
        tileA[j % NCORES].append(i)
    assert all(len(a) <= NSLOT for a in tileA), "cluster-2 per-core overflow"
    c = 0
    for i in idx_sl + c0_ovf:
        placed = False
        for d in range(NCORES):
            cc = (c + d) % NCORES
            if len(tileA[cc]) < NSLOT:
                tileA[cc].append(i)
                c = (cc + 1) % NCORES
                placed = True
                break
        assert placed, "tile A overflow"
    for c in range(NCORES):
        tileA[c] += [-1] * (NSLOT - len(tileA[c]))
        tileB[c] += [-1] * (NSLOT - len(tileB[c]))
    return tileA, tileB, cl


def _kpack(mat):
    """[512, C] -> [128, 4*C] with col = k*C + j (K-chunk packing for matmul)."""
    C = mat.shape[1]
    return np.ascontiguousarray(
        mat.reshape(4, 128, C).transpose(1, 0, 2).reshape(128, 4 * C))


def _host_arrays(inputs, tileA, tileB, cl):
    import ml_dtypes
    bf16 = ml_dtypes.bfloat16

    x = np.asarray(inputs["user_repr"], np.float32)
    t = np.asarray(inputs["targets"]).astype(np.int64)
    head_w = np.asarray(inputs["head_w"], np.float32)
    w1 = [np.asarray(inputs[f"tail_w1_{k}"], np.float32) for k in range(3)]
    w2 = [np.asarray(inputs[f"tail_w2_{k}"], np.float32) for k in range(3)]

    def pad_T(w, cols):
        out = np.zeros((w.shape[1], cols), np.float32)
        out[:, : w.shape[0]] = w.T
        return out

    # ---- shared (replicated) weights, packed into SBUF layout ----
    # head: col = (f*4 + k)*512 so each 512-col F-chunk is DMA-contiguous
    hwT = _kpack(pad_T(head_w, PH)).reshape(128, 4, 2, 512)\
        .transpose(0, 2, 1, 3).reshape(128, 4096).copy().astype(bf16)
    w1p = [_kpack(np.ascontiguousarray(w1[s].T)) for s in range(3)]  # [128, 4*hsz]
    w2_0T = pad_T(w2[0], P0).astype(bf16)                      # [128, 10240]

    def quarters(w, qpad):
        # [osz, hsz] -> [4, hsz, qpad]: osz split into 4 equal quarters,
        # each zero-padded to qpad (so the pad sits at each quarter's END)
        hsz = w.shape[1]
        qr = w.shape[0] // 4
        out = np.zeros((4, hsz, qpad), np.float32)
        out[:, :, :qr] = w.T.reshape(hsz, 4, qr).transpose(1, 0, 2)
        return out

    w2_1p = np.ascontiguousarray(
        quarters(w2[1], Q1).reshape(128, Q1)).astype(bf16)     # [128, 10240]
    # w2_2 packed dense: row = 8q + r; DMAed into SBUF partitions 32q + r
    w2_2p = np.ascontiguousarray(
        quarters(w2[2], Q2).reshape(32, Q2)).astype(bf16)      # [32, 12800]

    in_maps = []
    for c in range(NCORES):
        slots = tileA[c] + tileB[c]
        xs = np.zeros((256, D), np.float32)
        gr = np.zeros((256, D), np.float32)
        for s, i in enumerate(slots):
            if i < 0:
                continue
            xs[s] = x[i]
            ci = cl[i]
            gr[s] = head_w[int(t[i])] if ci == -1 else head_w[SHORT + ci]
        mA2 = np.array([1.0 if (i >= 0 and cl[i] == 2) else 0.0 for i in tileA[c]], np.float32)
        mA0 = np.array([1.0 if (i >= 0 and cl[i] == 0) else 0.0 for i in tileA[c]], np.float32)
        mB1 = np.array([1.0 if (i >= 0 and cl[i] == 1) else 0.0 for i in tileB[c]], np.float32)
        mB0 = np.array([1.0 if (i >= 0 and cl[i] == 0) else 0.0 for i in tileB[c]], np.float32)
        xA, xB = xs[:128], xs[128:]
        # masked transposed x for the four tail stationaries, K-chunk packed:
        # each block [128, 512] with col = k*128 + slot
        xm_blk = {}
        for key, mvec, xt in (("A2", mA2, xA), ("A0", mA0, xA),
                              ("B1", mB1, xB), ("B0", mB0, xB)):
            xm_blk[key] = _kpack(np.ascontiguousarray((xt * mvec[:, None]).T))
        # grT: gathered head rows, transposed + K-chunk packed like xT
        grT = _kpack(np.ascontiguousarray(gr.T))                  # [128, 1024]
        # gathered tail target rows, transposed: [hsz, 128] in 4 col-blocks
        g2T = np.zeros((128, 4 * 128), np.float32)
        for s, i in enumerate(tileA[c]):
            if i < 0:
                continue
            if cl[i] == 2:
                g2T[:8, s] = w2[2][int(t[i]) - CUT[2]]
            elif cl[i] == 0:
                g2T[:, 128 + s] = w2[0][int(t[i]) - CUT[0]]
        for s, i in enumerate(tileB[c]):
            if i < 0:
                continue
            if cl[i] == 1:
                g2T[:32, 256 + s] = w2[1][int(t[i]) - CUT[1]]
            elif cl[i] == 0:
                g2T[:, 384 + s] = w2[0][int(t[i]) - CUT[0]]
        # pad corrections: the last ring group is trimmed to the real column
        # count, so cluster-2/1 rows see no zero-pad exps; cluster-0 rows
        # still see P0 - osz0 of them
        corrA = -(P0 - OSZ[0]) * mA0 + (1.0 - mA2 - mA0)
        corrB = -(P0 - OSZ[0]) * mB0 + (1.0 - mB1 - mB0)
        mcpack = np.stack([mA2, mA0, mB1, mB0, corrA, corrB], axis=1)\
            .astype(np.float32)                                   # [128, 6]

        # one packed small-input tensor, consumption-ordered so the first
        # DMA splits carry exactly what the first projections need
        small = np.concatenate([
            w1p[2],                                  # [128, 32]   OFF_W12
            xm_blk["A2"],                            # [128, 512]  OFF_XA2
            w1p[1],                                  # [128, 128]  OFF_W11
            xm_blk["B1"],                            # [128, 512]  OFF_XB1
            w1p[0],                                  # [128, 512]  OFF_W10
            xm_blk["A0"],                            # [128, 512]  OFF_XA0
            xm_blk["B0"],                            # [128, 512]  OFF_XB0
            _kpack(np.ascontiguousarray(xs.T)),      # [128, 1024] OFF_XT
            grT,                                     # [128, 1024] OFF_GRT
            g2T,                                     # [128, 512]  OFF_G2T
            np.ones((128, 16), np.float32),          # [128, 16]   OFF_ONE
        ], axis=1)
        m = {
            "small": small.astype(bf16),
            "mcpack": mcpack,
            "hwT": hwT,
            "w2_0T": w2_0T, "w2_1p": w2_1p, "w2_2p": w2_2p,
        }
        in_maps.append(m)
    return in_maps


# ================================ device program ================================

def build_nc():
    from concourse import bacc, tile
    import concourse.mybir as mybir

    bf = mybir.dt.bfloat16
    f32 = mybir.dt.float32
    i16 = mybir.dt.int16
    AF = mybir.ActivationFunctionType
    ALU = mybir.AluOpType
    AX = mybir.AxisListType

    nc = bacc.Bacc("TRN2", target_bir_lowering=False, debug=False,
                   num_devices=NCORES)

    dp = nc.declare_dram_parameter
    d_small = dp("small", [128, SMALL_C], bf, False)
    d_mc = dp("mcpack", [128, 6], f32, False)
    d_hwT = dp("hwT", [128, 4 * PH], bf, False)
    d_w2_0T = dp("w2_0T", [HSZ[0], P0], bf, False)
    d_w2_1p = dp("w2_1p", [128, Q1], bf, False)
    d_w2_2p = dp("w2_2p", [32, Q2], bf, False)
    d_out = dp("out", [128, 4], f32, True)

    with tile.TileContext(nc) as tc:
        with tc.tile_pool(name="consts", bufs=1) as cp, \
             tc.tile_pool(name="acc", bufs=1) as ap_, \
             tc.tile_pool(name="scr", bufs=2) as sp, \
             tc.tile_pool(name="iscr", bufs=2) as ipool, \
             tc.tile_pool(name="dscr", bufs=2) as dpool:

            # pin the table set that holds BOTH Exp and Ln so the tail Ln
            # does not pay a ~2.7us table switch
            nc.scalar.add_instruction(mybir.InstLoadActFuncSet(
                name=nc.get_next_instruction_name(), ins=[], outs=[],
                act_func_set_id=6))

            # ------------- DMA issue (sync queue), consumption-ordered --------
            s_small = cp.tile([128, SMALL_C], bf, name="s_small", tag="s_small")

            def sm_load(lo, hi):
                nc.sync.dma_start(out=s_small[:, lo:hi], in_=d_small.ap()[:, lo:hi])

            def w22_tile(ch, eng=None):
                # 4 dense quarter-DMAs per chunk (partition-split APs corrupt
                # on HW).  The first two chunks ride the low-latency sync
                # queue; later ones go on the idle GPSIMD queue.
                cw = W22_G[ch] * 512
                off = sum(W22_G[:ch]) * 512
                t_ = cp.tile([128, cw], bf, name=f"s_w22_{ch}", tag=f"s_w22_{ch}")
                for g in range(4):
                    (eng or nc.gpsimd).dma_start(
                        out=t_[32 * g:32 * g + 8, :],
                        in_=d_w2_2p.ap()[8 * g:8 * g + 8, off:off + cw])
                return t_

            def w20_tile(ch):
                cw = W20_CH[ch]
                off = W20_CH[0] if ch else 0
                t_ = cp.tile([128, cw], bf, name=f"s_w20_{ch}", tag=f"s_w20_{ch}")
                nc.sync.dma_start(out=t_[:, :], in_=d_w2_0T.ap()[:, off:off + cw])
                return t_

            def w21_tile(ch):
                cw = W21_CH[ch]
                off = W21_CH[0] if ch else 0
                t_ = cp.tile([128, cw], bf, name=f"s_w21_{ch}", tag=f"s_w21_{ch}")
                nc.sync.dma_start(out=t_[:, :], in_=d_w2_1p.ap()[:, off:off + cw])
                return t_

            s_w22 = [None] * 6
            sm_load(0, 544)              # w1_2 + xmA2  -> h2 projection
            s_w22[1] = w22_tile(1, nc.sync)   # ring A t=5,6
            s_w22[2] = w22_tile(2, nc.sync)   # t=7..10
            sm_load(544, 1184)           # w1_1 + xmB1  -> h1 projection
            sm_load(1184, 2720)          # w1_0 + xmA0 + xmB0 -> h0 projection
            sm_load(2720, 3744)          # xT -> heads
            s_hwT = cp.tile([128, 4096], bf, name="s_hwT", tag="s_hwT")
            nc.sync.dma_start(out=s_hwT[:, :], in_=d_hwT.ap())
            s_w22[3] = w22_tile(3)       # t=11..14
            s_w22[4] = w22_tile(4)       # t=15..19
            s_w22[5] = w22_tile(5)       # t=20..24
            s_w20 = [w20_tile(0), None]  # overlay t=0..2
            s_w22[0] = w22_tile(0)       # overlay t=0..4
            s_w20[1] = w20_tile(1)       # overlay t=3..4
            sm_load(3744, SMALL_C)       # grT + g2T + ones -> p-products, dots
            s_w21 = [w21_tile(0), w21_tile(1)]
            s_mc = cp.tile([128, 6], f32, name="s_mc", tag="s_mc")
            nc.sync.dma_start(out=s_mc[:, :], in_=d_mc.ap())

            # ------------- views ----------------------------------------------
            def xm(m, k):        # masked-x chunk [128, 128]: m in A2,A0,B1,B0
                base = {0: OFF_XA2, 1: OFF_XA0, 2: OFF_XB1, 3: OFF_XB0}[m]
                return s_small[:, base + k * 128: base + k * 128 + 128]

            def xTk(k, sl):      # xT chunk [128, 128] for slot range sl
                return s_small[:, OFF_XT + k * 256 + sl * 128:
                               OFF_XT + k * 256 + sl * 128 + 128]

            def hwk(k, f):       # head weight chunk [128, 512]
                o = (f * 4 + k) * 512
                return s_hwT[:, o: o + 512]

            def w1k(seg, k):     # w1 seg (0:128 | 1:32 | 2:8) k-chunk
                base = {0: OFF_W10, 1: OFF_W11, 2: OFF_W12}[seg]
                w = HSZ[seg]
                return s_small[:, base + k * w: base + (k + 1) * w]

            s_xT = s_small[:, OFF_XT:OFF_XT + 1024]
            s_grT = s_small[:, OFF_GRT:OFF_GRT + 1024]
            s_g2T = s_small[:, OFF_G2T:OFF_G2T + 512]
            s_ones = s_small[:, OFF_ONE:OFF_ONE + 1]

            mA2_v, mA0_v = s_mc[:, 0:1], s_mc[:, 1:2]
            mB1_v, mB0_v = s_mc[:, 2:3], s_mc[:, 3:4]
            corrA_v, corrB_v = s_mc[:, 4:5], s_mc[:, 5:6]

            # accumulators / combine tiles
            accA = ap_.tile([128, ACC_COLS], f32, name="accA", tag="accA")
            accB = ap_.tile([128, ACC_COLS], f32, name="accB", tag="accB")
            accH = ap_.tile([128, 4], f32, name="accH", tag="accH")
            tgt4 = ap_.tile([128, 4], f32, name="tgt4", tag="tgt4")
            S4 = ap_.tile([128, 4], f32, name="S4", tag="S4")
            ln4 = ap_.tile([128, 4], f32, name="ln4", tag="ln4")
            out4 = ap_.tile([128, 4], f32, name="out4", tag="out4")
            tmp = [ap_.tile([128, 1], f32, name=f"tmp{i}", tag=f"tmp{i}")
                   for i in range(6)]
            s_h2a = ap_.tile([128, 128], bf, name="s_h2a", tag="s_h2a")
            s_h1b = ap_.tile([128, 128], bf, name="s_h1b", tag="s_h1b")
            s_h0a = ap_.tile([128, 128], bf, name="s_h0a", tag="s_h0a")
            s_h0b = ap_.tile([128, 128], bf, name="s_h0b", tag="s_h0b")

            with tc.tile_pool(name="psR", bufs=2, space="PSUM") as pr:
                # ---- early projections as ring-pool pseudo-groups ----
                # h2 matmuls + CASTs first: they gate the whole ring.  The
                # h1 projection is emitted AFTER the first ring group so the
                # first exp's engine-counter dependency doesn't include it.
                h21 = pr.tile([128, 256], f32, name="h21", tag="ring")
                for g in range(4):
                    for k in range(4):
                        nc.tensor.matmul(
                            h21[32 * g:32 * g + 8, 0:128], w1k(2, k), xm(0, k),
                            start=(k == 0), stop=(k == 3),
                            tile_position=(0, 32 * g))
                for g in range(4):
                    nc.vector.tensor_copy(s_h2a[32 * g:32 * g + 8, :],
                                          h21[32 * g:32 * g + 8, 0:128])

                def h1_proj():
                    for g in range(4):
                        for k in range(4):
                            nc.tensor.matmul(
                                h21[32 * g:32 * g + 32, 128:256], w1k(1, k), xm(2, k),
                                start=(k == 0), stop=(k == 3),
                                tile_position=(0, 32 * g))
                    for g in range(4):
                        nc.vector.tensor_copy(s_h1b[32 * g:32 * g + 32, :],
                                              h21[32 * g:32 * g + 32, 128:256])

                # ---- exp + per-slot row sum, split ACT / DVE ----
                def exp_group(pt, acc_ap, dve, fw=512):
                    if dve:
                        assert fw == 512
                        it = ipool.tile([128, GRP], i16, name="it", tag="it")
                        nc.vector.tensor_scalar(
                            it[:, :], pt[:, :], A16, B16C, ALU.mult, ALU.add)
                        # halve in 2x-mode bf16 TT, then 1x accum-reduce
                        bv = it[:, :].bitcast(bf)
                        ds = dpool.tile([128, GRP // 2], bf, name="ds", tag="ds")
                        nc.vector.tensor_add(ds[:, :], bv[:, 0:GRP // 2],
                                             bv[:, GRP // 2:GRP])
                        ds2 = dpool.tile([128, GRP // 2], bf, name="ds2", tag="ds2")
                        nc.vector.tensor_scalar(
                            ds2[:, :], ds[:, :], 1.0, None,
                            ALU.mult, ALU.add, accum_out=acc_ap)
                    else:
                        rsc = sp.tile([128, GRP], bf, name="rscr", tag="rscr")
                        src = pt[:, :] if fw == 512 else \
                            pt.rearrange("p (b e) -> p b e", b=4)[:, :, 0:fw]
                        dst = rsc[:, :] if fw == 512 else \
                            rsc.rearrange("p (b e) -> p b e", b=4)[:, :, 0:fw]
                        nc.scalar.activation(dst, src, AF.Exp, accum_out=acc_ap)

                # ring A: cluster-2 quarters + cluster-0 overflow (groups < G0).
                # Groups >= G0 (no w2_0 dependency) run first so the ring can
                # start as soon as the first w2_2 chunk lands.
                w22_bounds = np.cumsum([0] + W22_G).tolist()

                def ringA_group(t):
                    ch = next(i for i in range(len(W22_G))
                              if w22_bounds[i + 1] > t)
                    w = t - w22_bounds[ch]
                    # last group: only 12500 % 512 = 212 real cols per quarter
                    fw = 212 if t == G2 - 1 else 512
                    pt = pr.tile([128, GRP], f32, name="ringA", tag="ring")
                    for g in range(4):
                        nc.tensor.matmul(
                            pt[:, g * 512:g * 512 + fw],
                            s_h2a[32 * g:32 * g + 8, :],
                            s_w22[ch][32 * g:32 * g + 8, w * 512:w * 512 + fw],
                            start=True, stop=(t >= G0),
                            tile_position=(32 * g, 0))
                    if t < G0:
                        ch0 = 0 if t < 3 else 1
                        o0 = t * GRP - (0 if ch0 == 0 else W20_CH[0])
                        for g in range(4):
                            nc.tensor.matmul(
                                pt[:, g * 512:(g + 1) * 512],
                                s_h0a[:, :],
                                s_w20[ch0][:, o0 + g * 512:o0 + (g + 1) * 512],
                                start=False, stop=True)
                    exp_group(pt, accA[:, t:t + 1], ("A", t) in DVE_SET, fw)

                def head_tile(s):
                    # head logits + exp-sum for sample tile s; one ring slot,
                    # PE cost hides under ring ACT slack
                    hp = pr.tile([128, PH], f32, name=f"head_ps{s}", tag="ring")
                    for f in range(PH // 512):
                        for k in range(4):
                            nc.tensor.matmul(
                                hp[:, f * 512:(f + 1) * 512],
                                xTk(k, s), hwk(k, f),
                                start=(k == 0), stop=(k == 3))
                    hsc = sp.tile([128, PH], bf, name="hscr", tag="rscr")
                    nc.scalar.activation(hsc[:, :], hp[:, :], AF.Exp,
                                         accum_out=accH[:, 2 * s:2 * s + 1])

                for t in range(G0, G2):
                    if t == 6:
                        h1_proj()
                    elif t == 7:
                        # h0 projections as one ring-pool pseudo-group (their
                        # inputs land in DMA 4; only overlay groups need them)
                        h0t = pr.tile([128, 256], f32, name="h0t", tag="ring")
                        for k in range(4):
                            nc.tensor.matmul(h0t[:, 0:128], w1k(0, k), xm(1, k),
                                             start=(k == 0), stop=(k == 3))
                        for k in range(4):
                            nc.tensor.matmul(h0t[:, 128:256], w1k(0, k), xm(3, k),
                                             start=(k == 0), stop=(k == 3))
                        nc.vector.tensor_copy(s_h0a[:, :], h0t[:, 0:128])
                        nc.vector.tensor_copy(s_h0b[:, :], h0t[:, 128:256])
                    ringA_group(t)
                    if t == 14:
                        head_tile(0)
                    elif t == 16:
                        head_tile(1)

                # p-products for the target dots (the dot matmuls themselves
                # run after ring B starts)
                ph = sp.tile([128, 1024], bf, name="ph", tag="ph")
                nc.vector.tensor_mul(ph[:, :], s_xT[:, :], s_grT[:, :])
                p2 = sp.tile([128, 128], bf, name="p2", tag="p2")
                nc.vector.tensor_mul(p2[0:8, :], s_h2a[0:8, :], s_g2T[0:8, 0:128])
                p0a = sp.tile([128, 128], bf, name="p0a", tag="p0a")
                nc.vector.tensor_mul(p0a[:, :], s_h0a[:, :], s_g2T[:, 128:256])
                p1 = sp.tile([128, 128], bf, name="p1", tag="p1")
                nc.vector.tensor_mul(p1[0:32, :], s_h1b[0:32, :], s_g2T[0:32, 256:384])
                p0b = sp.tile([128, 128], bf, name="p0b", tag="p0b")
                nc.vector.tensor_mul(p0b[:, :], s_h0b[:, :], s_g2T[:, 384:512])

                def ringB_group(t):
                    ch = t // 10
                    w = t % 10
                    # last group: only 10000 % 512 = 272 real cols per quarter
                    fw = 272 if t == G1 - 1 else 512
                    pt = pr.tile([128, GRP], f32, name="ringB", tag="ring")
                    for g in range(4):
                        nc.tensor.matmul(
                            pt[:, g * 512:g * 512 + fw],
                            s_h1b[32 * g:32 * g + 32, :],
                            s_w21[ch][32 * g:32 * g + 32, w * 512:w * 512 + fw],
                            start=True, stop=(t >= G0),
                            tile_position=(32 * g, 0))
                    if t < G0:
                        ch0 = 0 if t < 3 else 1
                        o0 = t * GRP - (0 if ch0 == 0 else W20_CH[0])
                        for g in range(4):
                            nc.tensor.matmul(
                                pt[:, g * 512:(g + 1) * 512],
                                s_h0b[:, :],
                                s_w20[ch0][:, o0 + g * 512:o0 + (g + 1) * 512],
                                start=False, stop=True)
                    exp_group(pt, accB[:, t:t + 1], ("B", t) in DVE_SET, fw)

                for t in range(G0):
                    ringA_group(t)

                # ring-A side of the combine (DVE; runs while ring B exps)
                nc.vector.tensor_reduce(tmp[0][:, :], accA[:, 0:G2], axis=AX.X, op=ALU.add)
                nc.vector.tensor_reduce(tmp[1][:, :], accA[:, 0:G0], axis=AX.X, op=ALU.add)
                nc.vector.tensor_mul(tmp[0][:, :], tmp[0][:, :], mA2_v)
                nc.vector.tensor_mul(tmp[1][:, :], tmp[1][:, :], mA0_v)
                nc.vector.tensor_add(tmp[4][:, :], tmp[0][:, :], tmp[1][:, :])
                nc.vector.tensor_add(S4[:, 2:3], tmp[4][:, :], corrA_v)
                nc.vector.tensor_scalar_add(S4[:, 0:1], accH[:, 0:1], float(-(PH - 1003)))
                nc.vector.tensor_scalar_add(S4[:, 1:2], accH[:, 2:3], float(-(PH - 1003)))

                for t in range(7):
                    ringB_group(t)

                # target-logit dots: per-slot dot(u, v) = (u .* v)^T @ ones
                # (partition-dim contraction on the PE -> [slots, 1] PSUM);
                # one slot in ring B's light region
                dots_ps = pr.tile([128, GRP], f32, name="dots_ps", tag="ring")
                for k in range(4):
                    nc.tensor.matmul(dots_ps[:, 0:1], ph[:, k * 256:k * 256 + 128],
                                     s_ones[:, :], start=(k == 0), stop=(k == 3))
                for k in range(4):
                    nc.tensor.matmul(dots_ps[:, 1:2],
                                     ph[:, k * 256 + 128:k * 256 + 256],
                                     s_ones[:, :], start=(k == 0), stop=(k == 3))
                nc.tensor.matmul(dots_ps[:, 2:3], p2[0:8, :], s_ones[0:8, :],
                                 start=True, stop=False)
                nc.tensor.matmul(dots_ps[:, 2:3], p0a[:, :], s_ones[:, :],
                                 start=False, stop=True)
                nc.tensor.matmul(dots_ps[:, 3:4], p1[0:32, :], s_ones[0:32, :],
                                 start=True, stop=False)
                nc.tensor.matmul(dots_ps[:, 3:4], p0b[:, :], s_ones[:, :],
                                 start=False, stop=True)
                nc.vector.tensor_copy(tgt4[:, :], dots_ps[:, 0:4])

                for t in range(7, G1):
                    ringB_group(t)

            # ------------- combine (only ring-B accB remains) ----------------
            nc.vector.tensor_reduce(tmp[2][:, :], accB[:, 0:G1], axis=AX.X, op=ALU.add)
            nc.vector.tensor_reduce(tmp[3][:, :], accB[:, 0:G0], axis=AX.X, op=ALU.add)
            nc.vector.tensor_mul(tmp[2][:, :], tmp[2][:, :], mB1_v)
            nc.vector.tensor_mul(tmp[3][:, :], tmp[3][:, :], mB0_v)
            nc.vector.tensor_add(tmp[5][:, :], tmp[2][:, :], tmp[3][:, :])
            nc.vector.tensor_add(S4[:, 3:4], tmp[5][:, :], corrB_v)
            nc.scalar.activation(ln4[:, :], S4[:, :], AF.Ln)
            nc.vector.tensor_sub(out4[:, :], tgt4[:, :], ln4[:, :])
            nc.sync.dma_start(out=d_out.ap(), in_=out4[:, :])

    nc.compile()
    return nc


def _get_nc():
    global _BUILT
    if _BUILT is None:
        _BUILT = build_nc()
    return _BUILT


# ================================ entry point ================================

def _numpy_fallback(inputs):
    """Last-resort exact computation (only if the slot assignment misfits,
    which cannot happen for the deterministic problem inputs)."""
    x = np.asarray(inputs["user_repr"], np.float64)
    t = np.asarray(inputs["targets"]).astype(np.int64)
    head_w = np.asarray(inputs["head_w"], np.float64)
    rows = np.arange(x.shape[0])

    def lse_rows(logits):
        m = logits.max(axis=1, keepdims=True)
        return (np.log(np.exp(logits - m).sum(axis=1, keepdims=True)) + m)

    hl = x @ head_w.T
    head_lp = hl - lse_rows(hl)
    out = np.where(t < SHORT, head_lp[rows, np.minimum(t, SHORT - 1)], 0.0)
    for i in range(3):
        w1 = np.asarray(inputs[f"tail_w1_{i}"], np.float64)
        w2 = np.asarray(inputs[f"tail_w2_{i}"], np.float64)
        tl = (x @ w1.T) @ w2.T
        tail_lp = tl - lse_rows(tl)
        rel = np.clip(t - CUT[i], 0, CUT[i + 1] - CUT[i] - 1)
        val = head_lp[:, SHORT + i] + tail_lp[rows, rel]
        out = np.where((t >= CUT[i]) & (t < CUT[i + 1]), val, out)
    return out.astype(np.float32)


def _logit_bound_ok(inputs):
    """Cauchy-Schwarz bound on |tail logit| so the int16 Schraudolph path
    cannot overflow (needs |z| < ~80; typical max is ~12)."""
    x = np.asarray(inputs["user_repr"], np.float32)
    bound = 0.0
    for k in range(3):
        w1 = np.asarray(inputs[f"tail_w1_{k}"], np.float32)
        w2 = np.asarray(inputs[f"tail_w2_{k}"], np.float32)
        h = x @ w1.T
        hn = np.sqrt((h * h).sum(axis=1)).max()
        wn = np.sqrt((w2 * w2).sum(axis=1)).max()
        bound = max(bound, float(hn) * float(wn))
    return bound < 60.0


def kernel(**inputs):
    from concourse.bass_utils import run_bass_kernel_spmd

    targets = np.asarray(inputs["targets"]).astype(np.int64)
    try:
        tileA, tileB, cl = _assign(targets)
        if not _logit_bound_ok(inputs):
            return _numpy_fallback(inputs)
    except AssertionError:
        return _numpy_fallback(inputs)
    in_maps = _host_arrays(inputs, tileA, tileB, cl)
    nc = _get_nc()
    res = run_bass_kernel_spmd(nc, in_maps, core_ids=list(range(NCORES)))
    out = np.zeros(N, np.float32)
    for c in range(NCORES):
        o = res.results[c]["out"]   # [128, 4]
        for s, i in enumerate(tileA[c]):
            if i >= 0:
                out[i] = o[s, 0] + (o[s, 2] if cl[i] >= 0 else 0.0)
        for s, i in enumerate(tileB[c]):
            if i >= 0:
                out[i] = o[s, 1] + o[s, 3]
    return out


# revision 26
# speedup vs baseline: 1.0541x; 1.0377x over previous
"""AdaptiveSoftmaxProductHead.loss on 8 TRN2 NeuronCores (data-parallel).

Strategy
--------
Per-sample target log-prob = (head target logit - head logsumexp)
                           + [cluster: tail target logit - tail cluster logsumexp].

Host: assigns each of the 2048 samples to one of 8 cores, into one of two
128-slot tiles per core (tile A: cluster-2 + shortlist + cluster-0 overflow;
tile B: cluster-1 + cluster-0).  Gathers the per-sample target weight rows on
the host (pure data movement) so the device never needs data-dependent
indexing.  All device inputs are packed on the host into the exact SBUF
layout so each resident tensor loads with one (or few) large DMAs.

Device (identical SPMD program on every core, different data):
  - tail cluster logits [128 slots, osz] in 512-col PSUM chunks; 4 PE
    row-tiles (small-K packing) fill a 4-bank PSUM group.  Two tail clusters
    share one 128-slot tile via zero-masked stationaries and PSUM accumulate.
  - exp + per-slot row-sum of each [128, 2048] group is SPLIT between two
    engines: the ACT engine (true Exp with fused accumulator) and the DVE
    (Schraudolph bf16 exp: n = int16(z*128/ln2 + 16256); bitcast(n) ~ c*e^z,
    summed by a 4x-mode tensor_scalar with accum_out; the constant bias c
    is divided out in the same instruction).  This nearly doubles exp-sum
    throughput since the ACT engine alone was the critical resource.
  - head logits + exp-sums on ACT; target logits via per-slot dot products
    (VectorE products on GPSIMD + partition-dim ones-matmul on the PE).
  - ln + combine on device; host only unpermutes / adds the two parts.
"""

import numpy as np

# ---------------- problem constants (hardcoded; kernel.py is self-contained) ----
N, D = 2048, 512
SHORT = 1000
CUT = [1000, 10000, 50000, 100000]
OSZ = [9000, 40000, 50000]
HSZ = [128, 32, 8]
NCORES = 8
NSLOT = 128          # slots per tile
GRP = 2048           # columns per exp instruction (4 PSUM banks)
P0, P1, P2 = 10240, 40960, 51200   # padded tail column counts
PH = 1024                          # padded head columns
G0, G1, G2 = P0 // GRP, P1 // GRP, P2 // GRP   # 5, 20, 25 exp groups
Q1, Q2 = P1 // 4, P2 // 4          # per-quarter cols: 10240, 12800
ACC_COLS = 32
# w2_2 quarter chunks (in exp groups of 512 cols), group-aligned.
# chunk 0 = the G0 cluster-0-overlay groups (processed LAST in ring A);
# chunk 1 is small so the ring can start early.
W22_G = [5, 2, 4, 4, 5, 5]         # per-chunk group counts (sum = 25)
W21_CH = [10 * 512, 10 * 512]      # 5120, 5120
W20_CH = [3 * GRP, 2 * GRP]        # 6144, 4096

# bf16 Schraudolph exp on the DVE: n = int16(A16*z + B16), bitcast bf16.
# Mean multiplicative bias c (vs true e^z) is distribution-independent to
# ~2e-5; we divide it out in the reduce pass.  Calibrated midway between
# float->int truncation (c=1.037895) and round-to-nearest (c=1.040685)
# since the HW conversion mode costs at most 0.13% either way (harmless:
# it cancels to ~1e-4 relative in the final log-prob).
A16 = 184.66280009437495           # 128 / ln(2)
# 127*128, minus the bias correction folded in log-domain:
# 128*log2(1.0392913) = 7.11772
B16C = 16248.88228
KDVE = 15                          # exp groups handled by the DVE

# packed layout of the per-core "small" input tensor (bf16 columns)
OFF_W12, OFF_XA2 = 0, 32
OFF_W11, OFF_XB1 = 544, 672
OFF_W10, OFF_XA0, OFF_XB0 = 1184, 1696, 2208
OFF_XT, OFF_GRT, OFF_G2T, OFF_ONE = 2720, 3744, 4768, 5280
SMALL_C = 5296

_BUILT = None   # compiled Bass singleton


def _dve_groups():
    """Evenly spread KDVE of the 45 exp groups (consumption order) to the DVE.
    Excluded: the first group, the two partial (column-trimmed) groups, and
    the groups adjacent to the p-product / ring-A-combine DVE work bursts."""
    seq = [("A", t) for t in list(range(G0, G2)) + list(range(G0))] + \
          [("B", t) for t in range(G1)]
    partial = {("A", G2 - 1), ("B", G1 - 1)}
    skip = {0, 8, 9, 25, 26, 44}
    elig = [i for i, s in enumerate(seq)
            if s not in partial and i not in skip]
    idx = {elig[round(j * (len(elig) - 1) / (KDVE - 1))] for j in range(KDVE)}
    return {seq[i] for i in idx}


DVE_SET = _dve_groups()


# ================================ host-side prep ================================

def _assign(targets):
    """Per-core slot lists (tileA, tileB), each length-128 of sample idx or -1."""
    t = targets
    cl = np.where(t < CUT[0], -1,
                  np.where(t < CUT[1], 0, np.where(t < CUT[2], 1, 2)))
    idx_sl = np.nonzero(cl == -1)[0].tolist()
    idx_c = {k: np.nonzero(cl == k)[0].tolist() for k in range(3)}

    tileB = [[] for _ in range(NCORES)]
    for j, i in enumerate(idx_c[1]):
        tileB[j % NCORES].append(i)
    assert all(len(b) <= NSLOT for b in tileB), "cluster-1 per-core overflow"
    c0_ovf = []
    c = 0
    for i in idx_c[0]:
        placed = False
        for d in range(NCORES):
            cc = (c + d) % NCORES
            if len(tileB[cc]) < NSLOT:
                tileB[cc].append(i)
                c = (cc + 1) % NCORES
                placed = True
                break
        if not placed:
            c0_ovf.append(i)

    tileA = [[] for _ in range(NCORES)]
    for j, i in enumerate(idx_c[2]):
        tileA[j % NCORES].append(i)
    assert all(len(a) <= NSLOT for a in tileA), "cluster-2 per-core overflow"
    c = 0
    for i in idx_sl + c0_ovf:
        placed = False
        for d in range(NCORES):
            cc = (c + d) % NCORES
            if len(tileA[cc]) < NSLOT:
                tileA[cc].append(i)
                c = (cc + 1) % NCORES
                placed = True
                break
        assert placed, "tile A overflow"
    for c in range(NCORES):
        tileA[c] += [-1] * (NSLOT - len(tileA[c]))
        tileB[c] += [-1] * (NSLOT - len(tileB[c]))
    return tileA, tileB, cl


def _kpack(mat):
    """[512, C] -> [128, 4*C] with col = k*C + j (K-chunk packing for matmul)."""
    C = mat.shape[1]
    return np.ascontiguousarray(
        mat.reshape(4, 128, C).transpose(1, 0, 2).reshape(128, 4 * C))


def _host_arrays(inputs, tileA, tileB, cl):
    import ml_dtypes
    bf16 = ml_dtypes.bfloat16

    x = np.asarray(inputs["user_repr"], np.float32)
    t = np.asarray(inputs["targets"]).astype(np.int64)
    head_w = np.asarray(inputs["head_w"], np.float32)
    w1 = [np.asarray(inputs[f"tail_w1_{k}"], np.float32) for k in range(3)]
    w2 = [np.asarray(inputs[f"tail_w2_{k}"], np.float32) for k in range(3)]

    def pad_T(w, cols):
        out = np.zeros((w.shape[1], cols), np.float32)
        out[:, : w.shape[0]] = w.T
        return out

    # ---- shared (replicated) weights, packed into SBUF layout ----
    # head: col = (f*4 + k)*512 so each 512-col F-chunk is DMA-contiguous
    hwT = _kpack(pad_T(head_w, PH)).reshape(128, 4, 2, 512)\
        .transpose(0, 2, 1, 3).reshape(128, 4096).copy().astype(bf16)
    w1p = [_kpack(np.ascontiguousarray(w1[s].T)) for s in range(3)]  # [128, 4*hsz]
    w2_0T = pad_T(w2[0], P0).astype(bf16)                      # [128, 10240]

    def quarters(w, qpad):
        # [osz, hsz] -> [4, hsz, qpad]: osz split into 4 equal quarters,
        # each zero-padded to qpad (so the pad sits at each quarter's END)
        hsz = w.shape[1]
        qr = w.shape[0] // 4
        out = np.zeros((4, hsz, qpad), np.float32)
        out[:, :, :qr] = w.T.reshape(hsz, 4, qr).transpose(1, 0, 2)
        return out

    w2_1p = np.ascontiguousarray(
        quarters(w2[1], Q1).reshape(128, Q1)).astype(bf16)     # [128, 10240]
    # w2_2 packed dense: row = 8q + r; DMAed into SBUF partitions 32q + r
    w2_2p = np.ascontiguousarray(
        quarters(w2[2], Q2).reshape(32, Q2)).astype(bf16)      # [32, 12800]

    in_maps = []
    for c in range(NCORES):
        slots = tileA[c] + tileB[c]
        xs = np.zeros((256, D), np.float32)
        gr = np.zeros((256, D), np.float32)
        for s, i in enumerate(slots):
            if i < 0:
                continue
            xs[s] = x[i]
            ci = cl[i]
            gr[s] = head_w[int(t[i])] if ci == -1 else head_w[SHORT + ci]
        mA2 = np.array([1.0 if (i >= 0 and cl[i] == 2) else 0.0 for i in tileA[c]], np.float32)
        mA0 = np.array([1.0 if (i >= 0 and cl[i] == 0) else 0.0 for i in tileA[c]], np.float32)
        mB1 = np.array([1.0 if (i >= 0 and cl[i] == 1) else 0.0 for i in tileB[c]], np.float32)
        mB0 = np.array([1.0 if (i >= 0 and cl[i] == 0) else 0.0 for i in tileB[c]], np.float32)
        xA, xB = xs[:128], xs[128:]
        # masked transposed x for the four tail stationaries, K-chunk packed:
        # each block [128, 512] with col = k*128 + slot
        xm_blk = {}
        for key, mvec, xt in (("A2", mA2, xA), ("A0", mA0, xA),
                              ("B1", mB1, xB), ("B0", mB0, xB)):
            xm_blk[key] = _kpack(np.ascontiguousarray((xt * mvec[:, None]).T))
        # grT: gathered head rows, transposed + K-chunk packed like xT
        grT = _kpack(np.ascontiguousarray(gr.T))                  # [128, 1024]
        # gathered tail target rows, transposed: [hsz, 128] in 4 col-blocks
        g2T = np.zeros((128, 4 * 128), np.float32)
        for s, i in enumerate(tileA[c]):
            if i < 0:
                continue
            if cl[i] == 2:
                g2T[:8, s] = w2[2][int(t[i]) - CUT[2]]
            elif cl[i] == 0:
                g2T[:, 128 + s] = w2[0][int(t[i]) - CUT[0]]
        for s, i in enumerate(tileB[c]):
            if i < 0:
                continue
            if cl[i] == 1:
                g2T[:32, 256 + s] = w2[1][int(t[i]) - CUT[1]]
            elif cl[i] == 0:
                g2T[:, 384 + s] = w2[0][int(t[i]) - CUT[0]]
        # pad corrections: the last ring group is trimmed to the real column
        # count, so cluster-2/1 rows see no zero-pad exps; cluster-0 rows
        # still see P0 - osz0 of them
        corrA = -(P0 - OSZ[0]) * mA0 + (1.0 - mA2 - mA0)
        corrB = -(P0 - OSZ[0]) * mB0 + (1.0 - mB1 - mB0)
        mcpack = np.stack([mA2, mA0, mB1, mB0, corrA, corrB], axis=1)\
            .astype(np.float32)                                   # [128, 6]

        # one packed small-input tensor, consumption-ordered so the first
        # DMA splits carry exactly what the first projections need
        small = np.concatenate([
            w1p[2],                                  # [128, 32]   OFF_W12
            xm_blk["A2"],                            # [128, 512]  OFF_XA2
            w1p[1],                                  # [128, 128]  OFF_W11
            xm_blk["B1"],                            # [128, 512]  OFF_XB1
            w1p[0],                                  # [128, 512]  OFF_W10
            xm_blk["A0"],                            # [128, 512]  OFF_XA0
            xm_blk["B0"],                            # [128, 512]  OFF_XB0
            _kpack(np.ascontiguousarray(xs.T)),      # [128, 1024] OFF_XT
            grT,                                     # [128, 1024] OFF_GRT
            g2T,                                     # [128, 512]  OFF_G2T
            np.ones((128, 16), np.float32),          # [128, 16]   OFF_ONE
        ], axis=1)
        m = {
            "small": small.astype(bf16),
            "mcpack": mcpack,
            "hwT": hwT,
            "w2_0T": w2_0T, "w2_1p": w2_1p, "w2_2p": w2_2p,
        }
        in_maps.append(m)
    return in_maps


# ================================ device program ================================

def build_nc():
    from concourse import bacc, tile
    import concourse.mybir as mybir

    bf = mybir.dt.bfloat16
    f32 = mybir.dt.float32
    i16 = mybir.dt.int16
    AF = mybir.ActivationFunctionType
    ALU = mybir.AluOpType
    AX = mybir.AxisListType

    nc = bacc.Bacc("TRN2", target_bir_lowering=False, debug=False,
                   num_devices=NCORES)

    dp = nc.declare_dram_parameter
    d_small = dp("small", [128, SMALL_C], bf, False)
    d_mc = dp("mcpack", [128, 6], f32, False)
    d_hwT = dp("hwT", [128, 4 * PH], bf, False)
    d_w2_0T = dp("w2_0T", [HSZ[0], P0], bf, False)
    d_w2_1p = dp("w2_1p", [128, Q1], bf, False)
    d_w2_2p = dp("w2_2p", [32, Q2], bf, False)
    d_out = dp("out", [128, 4], f32, True)

    with tile.TileContext(nc) as tc:
        with tc.tile_pool(name="consts", bufs=1) as cp, \
             tc.tile_pool(name="acc", bufs=1) as ap_, \
             tc.tile_pool(name="scr", bufs=2) as sp, \
             tc.tile_pool(name="iscr", bufs=2) as ipool, \
             tc.tile_pool(name="dscr", bufs=2) as dpool:

            # pin the table set that holds BOTH Exp and Ln so the tail Ln
            # does not pay a ~2.7us table switch
            nc.scalar.add_instruction(mybir.InstLoadActFuncSet(
                name=nc.get_next_instruction_name(), ins=[], outs=[],
                act_func_set_id=6))

            # ------------- DMA issue (sync queue), consumption-ordered --------
            s_small = cp.tile([128, SMALL_C], bf, name="s_small", tag="s_small")

            def sm_load(lo, hi):
                nc.sync.dma_start(out=s_small[:, lo:hi], in_=d_small.ap()[:, lo:hi])

            def w22_tile(ch, eng=None):
                # 4 dense quarter-DMAs per chunk (partition-split APs corrupt
                # on HW).  The first two chunks ride the low-latency sync
                # queue; later ones go on the idle GPSIMD queue.
                cw = W22_G[ch] * 512
                off = sum(W22_G[:ch]) * 512
                t_ = cp.tile([128, cw], bf, name=f"s_w22_{ch}", tag=f"s_w22_{ch}")
                for g in range(4):
                    (eng or nc.gpsimd).dma_start(
                        out=t_[32 * g:32 * g + 8, :],
                        in_=d_w2_2p.ap()[8 * g:8 * g + 8, off:off + cw])
                return t_

            def w20_tile(ch):
                cw = W20_CH[ch]
                off = W20_CH[0] if ch else 0
                t_ = cp.tile([128, cw], bf, name=f"s_w20_{ch}", tag=f"s_w20_{ch}")
                nc.sync.dma_start(out=t_[:, :], in_=d_w2_0T.ap()[:, off:off + cw])
                return t_

            def w21_tile(ch):
                cw = W21_CH[ch]
                off = W21_CH[0] if ch else 0
                t_ = cp.tile([128, cw], bf, name=f"s_w21_{ch}", tag=f"s_w21_{ch}")
                nc.sync.dma_start(out=t_[:, :], in_=d_w2_1p.ap()[:, off:off + cw])
                return t_

            s_w22 = [None] * 6
            s_w22[1] = w22_tile(1, nc.sync)   # ring A t=5,6
            sm_load(0, 1184)             # w1_2+xmA2+w1_1+xmB1 -> h2/h1 proj
            s_w22[2] = w22_tile(2, nc.sync)   # t=7..10
            sm_load(1184, 2720)          # w1_0 + xmA0 + xmB0 -> h0 projection
            sm_load(2720, 3744)          # xT -> heads
            s_hwT = cp.tile([128, 4096], bf, name="s_hwT", tag="s_hwT")
            nc.sync.dma_start(out=s_hwT[:, :], in_=d_hwT.ap())
            sm_load(3744, SMALL_C)       # grT + g2T + ones -> p-products, dots
            s_w22[3] = w22_tile(3)       # t=11..14
            s_w22[4] = w22_tile(4)       # t=15..19
            s_w22[5] = w22_tile(5)       # t=20..24
            s_w20 = [w20_tile(0), None]  # overlay t=0..2
            s_w22[0] = w22_tile(0)       # overlay t=0..4
            s_w20[1] = w20_tile(1)       # overlay t=3..4
            s_w21 = [w21_tile(0), w21_tile(1)]
            s_mc = cp.tile([128, 6], f32, name="s_mc", tag="s_mc")
            nc.sync.dma_start(out=s_mc[:, :], in_=d_mc.ap())

            # ------------- views ----------------------------------------------
            def xm(m, k):        # masked-x chunk [128, 128]: m in A2,A0,B1,B0
                base = {0: OFF_XA2, 1: OFF_XA0, 2: OFF_XB1, 3: OFF_XB0}[m]
                return s_small[:, base + k * 128: base + k * 128 + 128]

            def xTk(k, sl):      # xT chunk [128, 128] for slot range sl
                return s_small[:, OFF_XT + k * 256 + sl * 128:
                               OFF_XT + k * 256 + sl * 128 + 128]

            def hwk(k, f):       # head weight chunk [128, 512]
                o = (f * 4 + k) * 512
                return s_hwT[:, o: o + 512]

            def w1k(seg, k):     # w1 seg (0:128 | 1:32 | 2:8) k-chunk
                base = {0: OFF_W10, 1: OFF_W11, 2: OFF_W12}[seg]
                w = HSZ[seg]
                return s_small[:, base + k * w: base + (k + 1) * w]

            s_xT = s_small[:, OFF_XT:OFF_XT + 1024]
            s_grT = s_small[:, OFF_GRT:OFF_GRT + 1024]
            s_g2T = s_small[:, OFF_G2T:OFF_G2T + 512]
            s_ones = s_small[:, OFF_ONE:OFF_ONE + 1]

            mA2_v, mA0_v = s_mc[:, 0:1], s_mc[:, 1:2]
            mB1_v, mB0_v = s_mc[:, 2:3], s_mc[:, 3:4]
            corrA_v, corrB_v = s_mc[:, 4:5], s_mc[:, 5:6]

            # accumulators / combine tiles
            accA = ap_.tile([128, ACC_COLS], f32, name="accA", tag="accA")
            accB = ap_.tile([128, ACC_COLS], f32, name="accB", tag="accB")
            accH = ap_.tile([128, 4], f32, name="accH", tag="accH")
            tgt4 = ap_.tile([128, 4], f32, name="tgt4", tag="tgt4")
            S4 = ap_.tile([128, 4], f32, name="S4", tag="S4")
            ln4 = ap_.tile([128, 4], f32, name="ln4", tag="ln4")
            out4 = ap_.tile([128, 4], f32, name="out4", tag="out4")
            tmp = [ap_.tile([128, 1], f32, name=f"tmp{i}", tag=f"tmp{i}")
                   for i in range(6)]
            s_h2a = ap_.tile([128, 128], bf, name="s_h2a", tag="s_h2a")
            s_h1b = ap_.tile([128, 128], bf, name="s_h1b", tag="s_h1b")
            s_h0a = ap_.tile([128, 128], bf, name="s_h0a", tag="s_h0a")
            s_h0b = ap_.tile([128, 128], bf, name="s_h0b", tag="s_h0b")

            with tc.tile_pool(name="psR", bufs=2, space="PSUM") as pr:
                # ---- early projections as ring-pool pseudo-groups ----
                # h2 + h1 matmuls (their inputs land in the first sm DMA);
                # each PSUM->SBUF copy is ONE full-tile CAST (unwritten rows
                # carry junk that no consumer ever reads)
                h21 = pr.tile([128, 256], f32, name="h21", tag="ring")
                for g in range(4):
                    for k in range(4):
                        nc.tensor.matmul(
                            h21[32 * g:32 * g + 8, 0:128], w1k(2, k), xm(0, k),
                            start=(k == 0), stop=(k == 3),
                            tile_position=(0, 32 * g))
                nc.vector.tensor_copy(s_h2a[:, :], h21[:, 0:128])
                for g in range(4):
                    for k in range(4):
                        nc.tensor.matmul(
                            h21[32 * g:32 * g + 32, 128:256], w1k(1, k), xm(2, k),
                            start=(k == 0), stop=(k == 3),
                            tile_position=(0, 32 * g))
                nc.vector.tensor_copy(s_h1b[:, :], h21[:, 128:256])

                # ---- exp + per-slot row sum, split ACT / DVE ----
                def exp_group(pt, acc_ap, dve, fw=512):
                    if dve:
                        assert fw == 512
                        it = ipool.tile([128, GRP], i16, name="it", tag="it")
                        nc.vector.tensor_scalar(
                            it[:, :], pt[:, :], A16, B16C, ALU.mult, ALU.add)
                        # halve in 2x-mode bf16 TT, then 1x accum-reduce
                        bv = it[:, :].bitcast(bf)
                        ds = dpool.tile([128, GRP // 2], bf, name="ds", tag="ds")
                        nc.vector.tensor_add(ds[:, :], bv[:, 0:GRP // 2],
                                             bv[:, GRP // 2:GRP])
                        ds2 = dpool.tile([128, GRP // 2], bf, name="ds2", tag="ds2")
                        nc.vector.tensor_scalar(
                            ds2[:, :], ds[:, :], 1.0, None,
                            ALU.mult, ALU.add, accum_out=acc_ap)
                    else:
                        rsc = sp.tile([128, GRP], bf, name="rscr", tag="rscr")
                        src = pt[:, :] if fw == 512 else \
                            pt.rearrange("p (b e) -> p b e", b=4)[:, :, 0:fw]
                        dst = rsc[:, :] if fw == 512 else \
                            rsc.rearrange("p (b e) -> p b e", b=4)[:, :, 0:fw]
                        nc.scalar.activation(dst, src, AF.Exp, accum_out=acc_ap)

                # ring A: cluster-2 quarters + cluster-0 overflow (groups < G0).
                # Groups >= G0 (no w2_0 dependency) run first so the ring can
                # start as soon as the first w2_2 chunk lands.
                w22_bounds = np.cumsum([0] + W22_G).tolist()

                def ringA_group(t):
                    ch = next(i for i in range(len(W22_G))
                              if w22_bounds[i + 1] > t)
                    w = t - w22_bounds[ch]
                    # last group: only 12500 % 512 = 212 real cols per quarter
                    fw = 212 if t == G2 - 1 else 512
                    pt = pr.tile([128, GRP], f32, name="ringA", tag="ring")
                    for g in range(4):
                        nc.tensor.matmul(
                            pt[:, g * 512:g * 512 + fw],
                            s_h2a[32 * g:32 * g + 8, :],
                            s_w22[ch][32 * g:32 * g + 8, w * 512:w * 512 + fw],
                            start=True, stop=(t >= G0),
                            tile_position=(32 * g, 0))
                    if t < G0:
                        ch0 = 0 if t < 3 else 1
                        o0 = t * GRP - (0 if ch0 == 0 else W20_CH[0])
                        for g in range(4):
                            nc.tensor.matmul(
                                pt[:, g * 512:(g + 1) * 512],
                                s_h0a[:, :],
                                s_w20[ch0][:, o0 + g * 512:o0 + (g + 1) * 512],
                                start=False, stop=True)
                    exp_group(pt, accA[:, t:t + 1], ("A", t) in DVE_SET, fw)

                def head_tile(s):
                    # head logits + exp-sum for sample tile s; one ring slot,
                    # PE cost hides under ring ACT slack
                    hp = pr.tile([128, PH], f32, name=f"head_ps{s}", tag="ring")
                    for f in range(PH // 512):
                        for k in range(4):
                            nc.tensor.matmul(
                                hp[:, f * 512:(f + 1) * 512],
                                xTk(k, s), hwk(k, f),
                                start=(k == 0), stop=(k == 3))
                    hsc = sp.tile([128, PH], bf, name="hscr", tag="rscr")
                    nc.scalar.activation(hsc[:, :], hp[:, :], AF.Exp,
                                         accum_out=accH[:, 2 * s:2 * s + 1])

                ph = sp.tile([128, 1024], bf, name="ph", tag="ph")
                p2 = sp.tile([128, 128], bf, name="p2", tag="p2")
                p0a = sp.tile([128, 128], bf, name="p0a", tag="p0a")
                p1 = sp.tile([128, 128], bf, name="p1", tag="p1")
                p0b = sp.tile([128, 128], bf, name="p0b", tag="p0b")

                for t in range(G0, G2):
                    if t == 9:
                        # h0 projections as one ring-pool pseudo-group (their
                        # inputs land in DMA 3; only overlay groups need them)
                        h0t = pr.tile([128, 256], f32, name="h0t", tag="ring")
                        for k in range(4):
                            nc.tensor.matmul(h0t[:, 0:128], w1k(0, k), xm(1, k),
                                             start=(k == 0), stop=(k == 3))
                        for k in range(4):
                            nc.tensor.matmul(h0t[:, 128:256], w1k(0, k), xm(3, k),
                                             start=(k == 0), stop=(k == 3))
                        nc.vector.tensor_copy(s_h0a[:, :], h0t[:, 0:128])
                        nc.vector.tensor_copy(s_h0b[:, :], h0t[:, 128:256])
                    elif t == 13:
                        # p-products for the target dots, in a light DVE
                        # stretch (the dot matmuls run after ring B starts)
                        nc.vector.tensor_mul(ph[:, :], s_xT[:, :], s_grT[:, :])
                        nc.vector.tensor_mul(p2[0:8, :], s_h2a[0:8, :],
                                             s_g2T[0:8, 0:128])
                        nc.vector.tensor_mul(p0a[:, :], s_h0a[:, :],
                                             s_g2T[:, 128:256])
                        nc.vector.tensor_mul(p1[0:32, :], s_h1b[0:32, :],
                                             s_g2T[0:32, 256:384])
                        nc.vector.tensor_mul(p0b[:, :], s_h0b[:, :],
                                             s_g2T[:, 384:512])
                    ringA_group(t)
                    if t == 14:
                        head_tile(0)
                    elif t == 16:
                        head_tile(1)

                def ringB_group(t):
                    ch = t // 10
                    w = t % 10
                    # last group: only 10000 % 512 = 272 real cols per quarter
                    fw = 272 if t == G1 - 1 else 512
                    pt = pr.tile([128, GRP], f32, name="ringB", tag="ring")
                    for g in range(4):
                        nc.tensor.matmul(
                            pt[:, g * 512:g * 512 + fw],
                            s_h1b[32 * g:32 * g + 32, :],
                            s_w21[ch][32 * g:32 * g + 32, w * 512:w * 512 + fw],
                            start=True, stop=(t >= G0),
                            tile_position=(32 * g, 0))
                    if t < G0:
                        ch0 = 0 if t < 3 else 1
                        o0 = t * GRP - (0 if ch0 == 0 else W20_CH[0])
                        for g in range(4):
                            nc.tensor.matmul(
                                pt[:, g * 512:(g + 1) * 512],
                                s_h0b[:, :],
                                s_w20[ch0][:, o0 + g * 512:o0 + (g + 1) * 512],
                                start=False, stop=True)
                    exp_group(pt, accB[:, t:t + 1], ("B", t) in DVE_SET, fw)

                for t in range(G0):
                    ringA_group(t)

                # ring-A side of the combine (DVE; runs while ring B exps)
                nc.vector.tensor_reduce(tmp[0][:, :], accA[:, 0:G2], axis=AX.X, op=ALU.add)
                nc.vector.tensor_reduce(tmp[1][:, :], accA[:, 0:G0], axis=AX.X, op=ALU.add)
                nc.vector.tensor_mul(tmp[0][:, :], tmp[0][:, :], mA2_v)
                nc.vector.tensor_mul(tmp[1][:, :], tmp[1][:, :], mA0_v)
                nc.vector.tensor_add(tmp[4][:, :], tmp[0][:, :], tmp[1][:, :])
                nc.vector.tensor_add(S4[:, 2:3], tmp[4][:, :], corrA_v)
                nc.vector.tensor_scalar_add(S4[:, 0:1], accH[:, 0:1], float(-(PH - 1003)))
                nc.vector.tensor_scalar_add(S4[:, 1:2], accH[:, 2:3], float(-(PH - 1003)))

                for t in range(7):
                    ringB_group(t)

                # target-logit dots: per-slot dot(u, v) = (u .* v)^T @ ones
                # (partition-dim contraction on the PE -> [slots, 1] PSUM);
                # one slot in ring B's light region
                dots_ps = pr.tile([128, GRP], f32, name="dots_ps", tag="ring")
                for k in range(4):
                    nc.tensor.matmul(dots_ps[:, 0:1], ph[:, k * 256:k * 256 + 128],
                                     s_ones[:, :], start=(k == 0), stop=(k == 3))
                for k in range(4):
                    nc.tensor.matmul(dots_ps[:, 1:2],
                                     ph[:, k * 256 + 128:k * 256 + 256],
                                     s_ones[:, :], start=(k == 0), stop=(k == 3))
                nc.tensor.matmul(dots_ps[:, 2:3], p2[0:8, :], s_ones[0:8, :],
                                 start=True, stop=False)
                nc.tensor.matmul(dots_ps[:, 2:3], p0a[:, :], s_ones[:, :],
                                 start=False, stop=True)
                nc.tensor.matmul(dots_ps[:, 3:4], p1[0:32, :], s_ones[0:32, :],
                                 start=True, stop=False)
                nc.tensor.matmul(dots_ps[:, 3:4], p0b[:, :], s_ones[:, :],
                                 start=False, stop=True)
                nc.vector.tensor_copy(tgt4[:, :], dots_ps[:, 0:4])

                for t in range(7, G1):
                    ringB_group(t)

            # ------------- combine (only ring-B accB remains) ----------------
            nc.vector.tensor_reduce(tmp[2][:, :], accB[:, 0:G1], axis=AX.X, op=ALU.add)
            nc.vector.tensor_reduce(tmp[3][:, :], accB[:, 0:G0], axis=AX.X, op=ALU.add)
            nc.vector.tensor_mul(tmp[2][:, :], tmp[2][:, :], mB1_v)
            nc.vector.tensor_mul(tmp[3][:, :], tmp[3][:, :], mB0_v)
            nc.vector.tensor_add(tmp[5][:, :], tmp[2][:, :], tmp[3][:, :])
            nc.vector.tensor_add(S4[:, 3:4], tmp[5][:, :], corrB_v)
            nc.scalar.activation(ln4[:, :], S4[:, :], AF.Ln)
            nc.vector.tensor_sub(out4[:, :], tgt4[:, :], ln4[:, :])
            nc.sync.dma_start(out=d_out.ap(), in_=out4[:, :])

    nc.compile()
    return nc


def _get_nc():
    global _BUILT
    if _BUILT is None:
        _BUILT = build_nc()
    return _BUILT


# ================================ entry point ================================

def _numpy_fallback(inputs):
    """Last-resort exact computation (only if the slot assignment misfits,
    which cannot happen for the deterministic problem inputs)."""
    x = np.asarray(inputs["user_repr"], np.float64)
    t = np.asarray(inputs["targets"]).astype(np.int64)
    head_w = np.asarray(inputs["head_w"], np.float64)
    rows = np.arange(x.shape[0])

    def lse_rows(logits):
        m = logits.max(axis=1, keepdims=True)
        return (np.log(np.exp(logits - m).sum(axis=1, keepdims=True)) + m)

    hl = x @ head_w.T
    head_lp = hl - lse_rows(hl)
    out = np.where(t < SHORT, head_lp[rows, np.minimum(t, SHORT - 1)], 0.0)
    for i in range(3):
        w1 = np.asarray(inputs[f"tail_w1_{i}"], np.float64)
        w2 = np.asarray(inputs[f"tail_w2_{i}"], np.float64)
        tl = (x @ w1.T) @ w2.T
        tail_lp = tl - lse_rows(tl)
        rel = np.clip(t - CUT[i], 0, CUT[i + 1] - CUT[i] - 1)
        val = head_lp[:, SHORT + i] + tail_lp[rows, rel]
        out = np.where((t >= CUT[i]) & (t < CUT[i + 1]), val, out)
    return out.astype(np.float32)


def _logit_bound_ok(inputs):
    """Cauchy-Schwarz bound on |tail logit| so the int16 Schraudolph path
    cannot overflow (needs |z| < ~80; typical max is ~12)."""
    x = np.asarray(inputs["user_repr"], np.float32)
    bound = 0.0
    for k in range(3):
        w1 = np.asarray(inputs[f"tail_w1_{k}"], np.float32)
        w2 = np.asarray(inputs[f"tail_w2_{k}"], np.float32)
        h = x @ w1.T
        hn = np.sqrt((h * h).sum(axis=1)).max()
        wn = np.sqrt((w2 * w2).sum(axis=1)).max()
        bound = max(bound, float(hn) * float(wn))
    return bound < 60.0


def kernel(**inputs):
    from concourse.bass_utils import run_bass_kernel_spmd

    targets = np.asarray(inputs["targets"]).astype(np.int64)
    try:
        tileA, tileB, cl = _assign(targets)
        if not _logit_bound_ok(inputs):
            return _numpy_fallback(inputs)
    except AssertionError:
        return _numpy_fallback(inputs)
    in_maps = _host_arrays(inputs, tileA, tileB, cl)
    nc = _get_nc()
    res = run_bass_kernel_spmd(nc, in_maps, core_ids=list(range(NCORES)))
    out = np.zeros(N, np.float32)
    for c in range(NCORES):
        o = res.results[c]["out"]   # [128, 4]
        for s, i in enumerate(tileA[c]):
            if i >= 0:
                out[i] = o[s, 0] + (o[s, 2] if cl[i] >= 0 else 0.0)
        for s, i in enumerate(tileB[c]):
            if i >= 0:
                out[i] = o[s, 1] + o[s, 3]
    return out
